# revision 3
# baseline (speedup 1.0000x reference)
"""nn_DecoderLayer (MLA attention + MoE routing) on 8 TRN2 NeuronCores.

Strategy:
  NEFF1 (attention): head-parallel — core c computes heads {2c, 2c+1}:
    replicated q_a/kv_a down-projections (feature-major, fp32r matmuls),
    per-head q_b/kv_b + RoPE (rotate-half folded into host-augmented
    weights), causal scoresT [k,q] layout, exp softmax without max
    subtraction (|scores| ~ 1.5), AV accumulate, partial o-projection.
    Host sums the 8 o-partials (expert-parallel style combine), adds
    residual, computes rmsnorm + router + top-4 routing in numpy.
  NEFF2 (MoE): expert-parallel — core c owns experts {2c, 2c+1}: gathered
    per-expert token batches (capacity CAP) through gate/up/silu/down with
    the combine weight folded into the activation; shared expert is
    token-parallel (core c handles tokens [256c, 256c+256)).
  Host scatters expert outputs back, adds shared + residual.

All matmuls fp32r (measured ~1.5e-4 rel err on HW at full PE rate).
"""
import math
import ml_dtypes
import numpy as np

import concourse.bacc as bacc
import concourse.mybir as mybir
import concourse.tile as tile
from concourse import bass_utils
from concourse.bass import ts
from concourse.masks import make_identity

# problem dims
S, H = 2048, 2048
NH, NOPE, ROPE, DV = 16, 128, 64, 128
DQK = NOPE + ROPE                  # 192
QR, KVR = 768, 512
E, KTOP, MI = 16, 4, 1024
SCALE = 2.5
EPS = 1e-6
ROPE_BASE = 10000.0

NC = 8                              # cores
HPC = NH // NC                      # heads/core = 2
EPC = E // NC                       # experts/core = 2
CAP = 640                           # per-expert token capacity (max seen ~547)
P = 128
NSTRIP = S // 512                   # 4 strips of 512 tokens

F32 = mybir.dt.float32
F32R = mybir.dt.float32r
BF16 = mybir.dt.bfloat16

Exp = mybir.ActivationFunctionType.Exp
Sqrt = mybir.ActivationFunctionType.Sqrt
Square = mybir.ActivationFunctionType.Square
Silu = mybir.ActivationFunctionType.Silu
Identity = mybir.ActivationFunctionType.Identity

_cache = {}

# profiling hooks (test.py sets TRACE=True; harness leaves it False)
TRACE = False
PROF = {}


def _run(nc, in_maps, name):
    if TRACE:
        res = bass_utils.run_bass_kernel_spmd(
            nc, in_maps, core_ids=list(range(NC)), trace=True, trace_cores=[0])
        it = res.instructions_and_trace or (None, None)
        PROF[name] = {"exec_time_ns": res.exec_time_ns, "trace": it[1],
                      "insts": it[0]}
        return res
    return bass_utils.run_bass_kernel_spmd(nc, in_maps,
                                           core_ids=list(range(NC)),
                                           trace=False)


# ---------------------------------------------------------------- NEFF 0
def build_neff0():
    """Token-sharded q_a/kv_a down-projections: core c handles tokens
    [256c, 256c+256). Outputs unscaled feature-major qn/kvn + sigma rows."""
    nc = bacc.Bacc("TRN2", num_devices=NC, debug=False)
    def inp(name, shape):
        return nc.dram_tensor(name, list(shape), F32, kind="ExternalInput").ap()

    xTs = inp("xTs", (P, 16, 256))                 # my tokens, feature-major
    qaw = inp("qaw", (P, 6, 16, P))                # lhsT: [p, m_tile, h_chunk, j]
    kvaw = inp("kvaw", (P, 4, 16, P))
    arow_i = inp("arow", (1, 256))                 # sum(x^2)/H + eps per token
    qn_out = nc.dram_tensor("qn_out", [10, P, 256], F32, kind="ExternalOutput").ap()
    sig_out = nc.dram_tensor("sig_out", [2, 1, 256], F32, kind="ExternalOutput").ap()

    with tile.TileContext(nc) as tc:
        with tc.tile_pool(name="p0", bufs=1) as p0, \
             tc.tile_pool(name="p02", bufs=2) as p02, \
             tc.tile_pool(name="ps0", bufs=2, space="PSUM") as ps0, \
             tc.tile_pool(name="ps0r", bufs=1, space="PSUM") as ps0r:
            ones_f = p0.tile([P, 1], F32)
            nc.vector.memset(ones_f, 1.0)
            ones_r = p0.tile([P, 1], F32R)
            nc.scalar.copy(ones_r, ones_f)
            qaw_sb = p0.tile([P, 6, 16, P], F32R)
            nc.sync.dma_start(out=qaw_sb, in_=qaw.bitcast(F32R))
            kvaw_sb = p0.tile([P, 4, 16, P], F32R)
            nc.sync.dma_start(out=kvaw_sb, in_=kvaw.bitcast(F32R))
            xs = p0.tile([P, 16, 256], F32R)
            nc.sync.dma_start(out=xs, in_=xTs.bitcast(F32R))
            a_row = p0.tile([1, 256], F32)
            nc.sync.dma_start(out=a_row, in_=arow_i)

            for kind in range(2):
                mt = 6 if kind == 0 else 4
                wsb = qaw_sb if kind == 0 else kvaw_sb
                fdim = QR if kind == 0 else KVR
                off = 0 if kind == 0 else 6
                msq_ps = ps0r.tile([1, 256], F32, tag="row")
                for m in range(mt):
                    mm_ps = ps0.tile([P, 256], F32, tag="mm")
                    for c in range(16):
                        nc.tensor.matmul(mm_ps, wsb[:, m, c, :], xs[:, c, :],
                                         start=(c == 0), stop=(c == 15))
                    raw = p02.tile([P, 256], F32R, tag="raw")
                    nc.scalar.copy(raw, mm_ps)
                    nc.sync.dma_start(out=qn_out[off + m], in_=raw.bitcast(F32))
                    sq = p02.tile([P, 256], F32R, tag="sq")
                    nc.scalar.activation(out=sq, in_=mm_ps, func=Square)
                    nc.tensor.matmul(msq_ps, ones_r, sq,
                                     start=(m == 0), stop=(m == mt - 1))
                msq_row = p02.tile([1, 256], F32, tag="msqr")
                nc.scalar.mul(msq_row, msq_ps, 1.0 / fdim)
                sig = p02.tile([1, 256], F32, tag="sig")
                nc.vector.scalar_tensor_tensor(
                    out=sig, in0=a_row, scalar=float(EPS), in1=msq_row,
                    op0=mybir.AluOpType.mult, op1=mybir.AluOpType.add)
                nc.scalar.activation(out=sig, in_=sig, func=Sqrt)
                nc.vector.reciprocal(sig, sig)
                nc.sync.dma_start(out=sig_out[kind], in_=sig)
    nc.compile()
    return nc


# ---------------------------------------------------------------- NEFF 1
def build_neff1():
    nc = bacc.Bacc("TRN2", num_devices=NC, debug=False)
    def inp(name, shape):
        return nc.dram_tensor(name, list(shape), F32, kind="ExternalInput").ap()

    qnT_i = inp("qnT", (10, P, S))                 # feature-major qn/kvn (unscaled)
    sig_i = inp("sig", (NSTRIP, 2, 1, 512))        # sigma rows
    qbw = inp("qbw", (P, 6, 512))                  # lhsT: [p, qr_chunk, m] (2 heads x 256)
    kvbw = inp("kvbw", (P, 4, 768))                # lhsT: [p, kvr_chunk, m] (2 heads x 384)
    ow = inp("ow", (P, HPC, H))                    # rhs: [p, c_chunk, h]
    cosT = inp("cosT", (ROPE, S))
    sinT = inp("sinT", (ROPE, S))
    masks = inp("masks", (P, 4, 512))              # tri01 keep-mask per diag offset
    o_part = nc.dram_tensor("o_part", [S, H], F32, kind="ExternalOutput").ap()

    with tile.TileContext(nc) as tc:
        with tc.tile_pool(name="const", bufs=1) as cpool, \
             tc.tile_pool(name="dram", bufs=1, space="DRAM") as dr:

            ones_f = cpool.tile([P, 1], F32)
            nc.vector.memset(ones_f, 1.0)
            ones_r = cpool.tile([P, 1], F32R)
            nc.scalar.copy(ones_r, ones_f)
            eps_t = cpool.tile([P, 1], F32)
            nc.vector.memset(eps_t, EPS)
            ident = cpool.tile([P, P], F32)
            make_identity(nc, ident)

            # DRAM scratch
            rcp_d = dr.tile([NSTRIP, HPC, 1, 512], F32)     # softmax denom bounce

            # ---------------- Windows B + C under the resident pool
            with tc.tile_pool(name="res", bufs=1) as res:
              qf_n = res.tile([P, HPC, S], F32R)
              qf_r = res.tile([64, HPC, S], F32R)
              kf_n = res.tile([P, HPC, S], F32R)
              kf_r = res.tile([64, HPC, S], F32R)
              v_sb = res.tile([P, HPC, 16, DV], F32R)
              # ---------------- Window B: q_b / kv_b + RoPE -> resident qf/kf/v
              with tc.tile_pool(name="pb", bufs=1) as pb, \
                   tc.tile_pool(name="pb2", bufs=2) as pb2, \
                   tc.tile_pool(name="pb1", bufs=1) as pb1, \
                   tc.tile_pool(name="psB", bufs=1, space="PSUM") as psB:
                  qbw_sb = pb.tile([P, 6, 512], F32R)
                  nc.sync.dma_start(out=qbw_sb, in_=qbw.bitcast(F32R))
                  kvbw_sb = pb.tile([P, 4, 768], F32R)
                  nc.sync.dma_start(out=kvbw_sb, in_=kvbw.bitcast(F32R))
                  cos_sb = pb.tile([ROPE, S], F32)
                  nc.sync.dma_start(out=cos_sb, in_=cosT)
                  sin_sb = pb.tile([ROPE, S], F32)
                  nc.sync.dma_start(out=sin_sb, in_=sinT)

                  for s in range(NSTRIP):
                      sl = slice(512 * s, 512 * (s + 1))
                      qn_t = pb2.tile([P, 10, 512], F32R, tag="qnt")
                      nc.sync.dma_start(out=qn_t,
                                        in_=qnT_i[:, :, sl].rearrange(
                                            "c p j -> p c j").bitcast(F32R))
                      sbcs = []
                      for kind in range(2):
                          sbc = pb1.tile([P, 512], F32, tag=f"sbc{kind}",
                                         name=f"sbc{kind}")
                          nc.sync.dma_start(out=sbc,
                                            in_=sig_i[s, kind].broadcast_to((P, 512)))
                          sbcs.append(sbc)
                      qn = [qn_t[:, c, :] for c in range(6)]
                      kvn = [qn_t[:, 6 + c, :] for c in range(4)]
                      for hi in range(HPC):
                          for side in range(2):  # 0: q, 1: k/v
                              if side == 0:
                                  wsb, chunks, base, nmt = qbw_sb, qn, 256 * hi, 6
                                  dn, rr, sbc = qf_n, qf_r, sbcs[0]
                              else:
                                  wsb, chunks, base, nmt = kvbw_sb, kvn, 384 * hi, 4
                                  dn, rr, sbc = kf_n, kf_r, sbcs[1]
                              ps_n = psB.tile([P, 512], F32, tag="bn")
                              ps_ro = psB.tile([64, 512], F32, tag="bro")
                              ps_rt = psB.tile([64, 512], F32, tag="brt")
                              for c in range(nmt):
                                  st, sp = (c == 0), (c == nmt - 1)
                                  nc.tensor.matmul(ps_n, wsb[:, c, base:base + 128],
                                                   chunks[c], start=st, stop=sp)
                                  nc.tensor.matmul(ps_ro,
                                                   wsb[:, c, base + 128:base + 192],
                                                   chunks[c], start=st, stop=sp)
                                  nc.tensor.matmul(ps_rt,
                                                   wsb[:, c, base + 192:base + 256],
                                                   chunks[c], start=st, stop=sp)
                              nc.vector.tensor_mul(dn[:, hi, sl], ps_n, sbc)
                              t1 = pb1.tile([64, 512], F32, tag="t1")
                              nc.vector.tensor_mul(t1, ps_ro, cos_sb[:, sl])
                              t2 = pb1.tile([64, 512], F32, tag="t2")
                              nc.vector.tensor_mul(t2, ps_rt, sin_sb[:, sl])
                              nc.vector.tensor_add(t1, t1, t2)
                              nc.vector.tensor_mul(rr[:, hi, sl], t1, sbc[0:64, :])
                              if side == 1:
                                  # v feature-major + sigma, then PE transpose
                                  ps_v = psB.tile([P, 512], F32, tag="bv")
                                  for c in range(4):
                                      nc.tensor.matmul(
                                          ps_v, kvbw_sb[:, c, base + 256:base + 384],
                                          chunks[c], start=(c == 0), stop=(c == 3))
                                  v_fm = pb1.tile([P, 512], F32, tag="vfm")
                                  nc.vector.tensor_mul(v_fm, ps_v, sbc)
                                  for t in range(4):
                                      ps_t = psB.tile([P, P], F32, tag="bt")
                                      nc.tensor.transpose(ps_t, v_fm[:, ts(t, P)],
                                                          ident)
                                      nc.scalar.copy(v_sb[:, hi, 4 * s + t, :], ps_t)

              # ---------------- Window C: attention + o-projection
              with tc.tile_pool(name="pc", bufs=1) as pc, \
                   tc.tile_pool(name="pc2", bufs=2) as pc2, \
                   tc.tile_pool(name="pc3", bufs=3) as pc3, \
                   tc.tile_pool(name="psC", bufs=2, space="PSUM") as psC, \
                   tc.tile_pool(name="psD", bufs=2, space="PSUM") as psD:
                  mask_sb = pc.tile([P, 4, 512], F32)
                  nc.sync.dma_start(out=mask_sb, in_=masks)
                  ow_sb = pc.tile([P, HPC, H], F32R)
                  nc.sync.dma_start(out=ow_sb, in_=ow.bitcast(F32R))

                  for s in range(NSTRIP):
                      sl = slice(512 * s, 512 * (s + 1))
                      ctx_sb = pc2.tile([P, HPC, 512], F32R, tag="ctx")
                      for hi in range(HPC):
                          ps_ctx = psD.tile([P, 512], F32, tag="ctx")
                          ps_den = psD.tile([1, 512], F32, tag="den")
                          nkc = 4 * s + 4
                          for kc in range(nkc):
                              ps_sc = psC.tile([P, 512], F32, tag="sc")
                              nc.tensor.matmul(ps_sc, kf_n[:, hi, ts(kc, P)],
                                               qf_n[:, hi, sl],
                                               start=True, stop=False)
                              nc.tensor.matmul(ps_sc, kf_r[:, hi, ts(kc, P)],
                                               qf_r[:, hi, sl],
                                               start=False, stop=True)
                              att = pc3.tile([P, 512], F32R, tag="att")
                              nc.scalar.activation(out=att, in_=ps_sc, func=Exp)
                              if kc >= 4 * s:
                                  nc.vector.tensor_mul(att, att.bitcast(F32),
                                                       mask_sb[:, kc - 4 * s, :])
                              nc.tensor.matmul(ps_den, ones_r, att,
                                               start=(kc == 0), stop=(kc == nkc - 1))
                              nc.tensor.matmul(ps_ctx, v_sb[:, hi, kc, :], att,
                                               start=(kc == 0), stop=(kc == nkc - 1))
                          den = pc2.tile([1, 512], F32, tag="den")
                          nc.scalar.copy(den, ps_den)
                          rcp = pc2.tile([1, 512], F32, tag="rcp")
                          nc.vector.reciprocal(rcp, den)
                          nc.sync.dma_start(out=rcp_d[s, hi], in_=rcp)
                          rcp_bc = pc2.tile([P, 512], F32, tag="rbc")
                          nc.sync.dma_start(out=rcp_bc,
                                            in_=rcp_d[s, hi].broadcast_to((P, 512)))
                          nc.vector.tensor_mul(ctx_sb[:, hi, :], ps_ctx, rcp_bc)
                      # o-projection for this strip (token-major out)
                      for t in range(4):
                          o_sb = pc2.tile([P, H], F32, tag="osb")
                          for hs in range(4):
                              ps_o = psC.tile([P, 512], F32, tag="o")
                              for cc in range(HPC):
                                  nc.tensor.matmul(ps_o, ctx_sb[:, cc, ts(t, P)],
                                                   ow_sb[:, cc, ts(hs, 512)],
                                                   start=(cc == 0), stop=(cc == HPC - 1))
                              nc.scalar.copy(o_sb[:, ts(hs, 512)], ps_o)
                          nc.sync.dma_start(
                              out=o_part.rearrange("(T p) h -> p T h", p=P)[:, 4 * s + t, :],
                              in_=o_sb)
    nc.compile()
    return nc


# ---------------------------------------------------------------- NEFF 2
def build_neff2():
    nc = bacc.Bacc("TRN2", num_devices=NC, debug=False)
    def inp(name, shape):
        return nc.dram_tensor(name, list(shape), F32, kind="ExternalInput").ap()

    xe = inp("xe", (EPC, 16, P, CAP))        # gathered expert tokens, feature-major
    gw = inp("gw", (EPC, 8, P, 2048))        # gate lhsT prepack
    uw = inp("uw", (EPC, 8, P, 2048))
    dw = inp("dw", (EPC, 8, P, 2048))        # down weights [m_chunk, p, h]
    wrow = inp("wrow", (EPC, 1, CAP))        # combine weights (row layout)
    h2t = inp("h2t", (16, P, 256))           # my 256 tokens, feature-major
    sgw = inp("sgw", (8, P, 2048))
    suw = inp("suw", (8, P, 2048))
    sdw = inp("sdw", (8, P, 2048))
    yrT = nc.dram_tensor("yrT", [EPC, 16, P, CAP], F32, kind="ExternalOutput").ap()
    yshT = nc.dram_tensor("yshT", [16, P, 256], F32, kind="ExternalOutput").ap()

    NS = CAP // 320  # 2 strips of 320

    with tile.TileContext(nc) as tc:
        with tc.tile_pool(name="p1", bufs=1) as p1, \
             tc.tile_pool(name="pw", bufs=2) as pw, \
             tc.tile_pool(name="pact", bufs=1) as pact, \
             tc.tile_pool(name="py", bufs=2) as py, \
             tc.tile_pool(name="ps", bufs=2, space="PSUM") as ps:

            for i in range(EPC):
                xe_sb = p1.tile([P, 16, CAP], F32R, tag="xe")
                nc.sync.dma_start(out=xe_sb, in_=xe[i].rearrange(
                    "hc p t -> p hc t").bitcast(F32R))
                w_bc = pact.tile([P, CAP], F32, tag="wbc")
                nc.sync.dma_start(out=w_bc, in_=wrow[i].broadcast_to((P, CAP)))
                act = pact.tile([P, 8, CAP], F32R, tag="act")
                for t in range(8):
                    g_w = pw.tile([P, 16, P], F32R, tag="gw")
                    nc.sync.dma_start(out=g_w, in_=gw[i, t].rearrange(
                        "p (hc j) -> p hc j", j=P).bitcast(F32R))
                    u_w = pw.tile([P, 16, P], F32R, tag="uw")
                    nc.sync.dma_start(out=u_w, in_=uw[i, t].rearrange(
                        "p (hc j) -> p hc j", j=P).bitcast(F32R))
                    for half in range(NS):
                        hsl = slice(320 * half, 320 * (half + 1))
                        ps_g = ps.tile([P, 320], F32, tag="g")
                        ps_u = ps.tile([P, 320], F32, tag="u")
                        for hc in range(16):
                            st, sp = (hc == 0), (hc == 15)
                            nc.tensor.matmul(ps_g, g_w[:, hc, :], xe_sb[:, hc, hsl],
                                             start=st, stop=sp)
                            nc.tensor.matmul(ps_u, u_w[:, hc, :], xe_sb[:, hc, hsl],
                                             start=st, stop=sp)
                        sil = pw.tile([P, 320], F32, tag="sil")
                        nc.scalar.activation(out=sil, in_=ps_g, func=Silu)
                        nc.vector.tensor_mul(act[:, t, hsl], sil, ps_u)
                # fold combine weight into act
                for t in range(8):
                    nc.vector.tensor_mul(act[:, t, :], act[:, t, :], w_bc)
                # down: feature-major out yT [h_tile, tok]
                d_w = p1.tile([P, 8, 2048], F32R, tag="dwr")
                nc.sync.dma_start(out=d_w,
                                  in_=dw[i].rearrange("c p h -> p c h").bitcast(F32R))
                for ht in range(16):
                    y_sb = py.tile([P, CAP], F32, tag="ysb")
                    for half in range(NS):
                        hsl = slice(320 * half, 320 * (half + 1))
                        ps_y = ps.tile([P, 320], F32, tag="y")
                        for mc in range(8):
                            nc.tensor.matmul(ps_y, d_w[:, mc, ts(ht, P)],
                                             act[:, mc, hsl],
                                             start=(mc == 0), stop=(mc == 7))
                        nc.scalar.copy(y_sb[:, hsl], ps_y)
                    nc.sync.dma_start(out=yrT[i, ht], in_=y_sb)

            # ---------------- shared expert (my 256 tokens)
            h2_sb = p1.tile([P, 16, 256], F32R, tag="h2")
            nc.sync.dma_start(out=h2_sb, in_=h2t.rearrange(
                "hc p t -> p hc t").bitcast(F32R))
            acts_full = pact.tile([P, 8, CAP], F32R, tag="act")
            acts = acts_full[:, :, 0:256]
            for t in range(8):
                g_w = pw.tile([P, 16, P], F32R, tag="gw")
                nc.sync.dma_start(out=g_w, in_=sgw[t].rearrange(
                    "p (hc j) -> p hc j", j=P).bitcast(F32R))
                u_w = pw.tile([P, 16, P], F32R, tag="uw")
                nc.sync.dma_start(out=u_w, in_=suw[t].rearrange(
                    "p (hc j) -> p hc j", j=P).bitcast(F32R))
                ps_g = ps.tile([P, 256], F32, tag="g")
                ps_u = ps.tile([P, 256], F32, tag="u")
                for hc in range(16):
                    st, sp = (hc == 0), (hc == 15)
                    nc.tensor.matmul(ps_g, g_w[:, hc, :], h2_sb[:, hc, :],
                                     start=st, stop=sp)
                    nc.tensor.matmul(ps_u, u_w[:, hc, :], h2_sb[:, hc, :],
                                     start=st, stop=sp)
                sil = pw.tile([P, 256], F32, tag="ssil")
                nc.scalar.activation(out=sil, in_=ps_g, func=Silu)
                nc.vector.tensor_mul(acts[:, t, :], sil, ps_u)
            d_w = p1.tile([P, 8, 2048], F32R, tag="dwr")
            nc.sync.dma_start(out=d_w,
                              in_=sdw.rearrange("c p h -> p c h").bitcast(F32R))
            for ht in range(16):
                ps_y = ps.tile([P, 256], F32, tag="y")
                for mc in range(8):
                    nc.tensor.matmul(ps_y, d_w[:, mc, ts(ht, P)],
                                     acts[:, mc, :],
                                     start=(mc == 0), stop=(mc == 7))
                y_sb = py.tile([P, 256], F32, tag="sysb")
                nc.scalar.copy(y_sb, ps_y)
                nc.sync.dma_start(out=yshT[ht], in_=y_sb)
    nc.compile()
    return nc


# ---------------------------------------------------------------- host prep
def _rope_tables():
    inv = 1.0 / (ROPE_BASE ** (np.arange(0, ROPE, 2, dtype=np.float64) / ROPE))
    t = np.arange(S, dtype=np.float64)
    f = t[:, None] * inv[None, :]
    emb = np.concatenate([f, f], axis=-1)          # [S, 64]
    return (np.cos(emb).T.astype(np.float32).copy(),
            np.sin(emb).T.astype(np.float32).copy())


def _lhsT_prepack(wT, mtiles):
    """wT [K, M] -> [P, mtiles, K//P, P]: SBUF-image for resident lhsT tiles."""
    Kd, Md = wT.shape
    assert Md == mtiles * P and Kd % P == 0
    return np.ascontiguousarray(
        wT.reshape(Kd // P, P, mtiles, P).transpose(1, 2, 0, 3))


def _lhsT_prepack2(wT, mtiles):
    """wT [K, M] -> [mtiles, P, K]: per-m-tile contiguous DMA layout.

    Tile t, flattened [P, K] with per-partition layout (hc, j):
    A[t, p, 128*hc + j] = wT[128*hc + p, 128*t + j].
    """
    Kd, Md = wT.shape
    assert Md == mtiles * P and Kd % P == 0
    return np.ascontiguousarray(
        wT.reshape(Kd // P, P, mtiles, P).transpose(2, 1, 0, 3).reshape(
            mtiles, P, Kd))


def _neff0_inputs(x, w):
    ln1 = w["ln1_w"]
    xT = x.T.astype(np.float32)                                   # [H, S]
    WqT = (w["q_a_w"] * ln1[None, :]).T.astype(np.float32)        # [H, QR]
    WkvT = (w["kv_a_w"] * ln1[None, :]).T.astype(np.float32)      # [H, KVR]
    qaw = _lhsT_prepack(WqT, 6)
    kvaw = _lhsT_prepack(WkvT, 4)
    arow = ((x * x).sum(-1) / H + EPS).astype(np.float32)         # [S]
    per_core = []
    for c in range(NC):
        rows = slice(256 * c, 256 * (c + 1))
        xTs = np.ascontiguousarray(xT[:, rows].reshape(16, P, 256)
                                   .transpose(1, 0, 2))
        per_core.append({"xTs": xTs, "qaw": qaw, "kvaw": kvaw,
                         "arow": arow[rows].reshape(1, 256)})
    return per_core


def _neff1_inputs(w, qnT, sig):
    qb = (w["q_b_w"] * w["q_a_ln"][None, :]).astype(np.float32)   # [NH*DQK, QR]
    kvb = (w["kv_b_w"] * w["kv_a_ln"][None, :]).astype(np.float32)  # [NH*320, KVR]
    sc = 1.0 / math.sqrt(DQK)
    cosT, sinT = _rope_tables()

    masks = np.zeros((P, 4, 512), np.float32)
    pp, jj = np.meshgrid(np.arange(P), np.arange(512), indexing="ij")
    for cl in range(4):
        masks[:, cl, :] = (P * cl + pp <= jj).astype(np.float32)

    per_core = []
    for c in range(NC):
        heads = [HPC * c + i for i in range(HPC)]
        # q_b augmented: per head rows [nope 128 | rope 64 | rot 64], scaled by sc
        qrows = []
        for h in heads:
            blk = qb[h * DQK:(h + 1) * DQK] * sc                   # [192, QR]
            nope, rope = blk[:NOPE], blk[NOPE:]
            rot = np.concatenate([-rope[32:], rope[:32]], axis=0)
            qrows.append(np.concatenate([nope, rope, rot], axis=0))  # [256, QR]
        qaug = np.concatenate(qrows, axis=0)                       # [512, QR]
        qbw = np.ascontiguousarray(qaug.T.reshape(6, P, 512).transpose(1, 0, 2))

        kvrows = []
        for h in heads:
            blk = kvb[h * 320:(h + 1) * 320]                       # [320, KVR]
            kn, kr, vv = blk[:NOPE], blk[NOPE:DQK], blk[DQK:]
            krot = np.concatenate([-kr[32:], kr[:32]], axis=0)
            kvrows.append(np.concatenate([kn, kr, krot, vv], axis=0))  # [384, KVR]
        kvaug = np.concatenate(kvrows, axis=0)                     # [768, KVR]
        kvbw = np.ascontiguousarray(kvaug.T.reshape(4, P, 768).transpose(1, 0, 2))

        ocols = np.concatenate([w["o_w"][:, h * DV:(h + 1) * DV] for h in heads],
                               axis=1)                             # [H, 256]
        owp = np.ascontiguousarray(ocols.T.reshape(HPC, P, H).transpose(1, 0, 2))

        per_core.append({
            "qnT": qnT, "sig": sig,
            "qbw": qbw.astype(np.float32), "kvbw": kvbw.astype(np.float32),
            "ow": owp.astype(np.float32), "cosT": cosT, "sinT": sinT,
            "masks": masks,
        })
    return per_core


def _route(h2ln, w):
    """Top-4 routing in numpy (fp32, matches jax semantics for these gaps)."""
    logits = h2ln @ w["router_w"].T.astype(np.float32) + w["router_b"][None, :]
    probs = 1.0 / (1.0 + np.exp(-logits))
    order = np.argsort(-probs, axis=-1, kind="stable")[:, :KTOP]
    topv = np.take_along_axis(probs, order, axis=-1)
    wts = topv / (topv.sum(-1, keepdims=True) + 1e-9) * SCALE
    return order, wts


def _neff2_inputs(h2, h2ln, w, order, wts):
    """h2: rmsnorm w/o ln2 (expert input pre-ln2-fold); h2ln unused here."""
    ln2 = w["ln2_w"]
    idx_lists, wt_lists = [], []
    for e in range(E):
        tok, kk = np.where(order == e)
        idx_lists.append(tok)
        wt_lists.append(wts[tok, kk])

    h2T = np.ascontiguousarray(h2.T)                        # [H, S]
    per_core = []
    spill = []                                              # (expert, tok, wt) overflow
    sgw = _lhsT_prepack2((w["sg_w"] * ln2[None, :]).T.astype(np.float32), 8)
    suw = _lhsT_prepack2((w["su_w"] * ln2[None, :]).T.astype(np.float32), 8)
    sdw = np.ascontiguousarray(w["sd_w"].T.reshape(8, P, H)).astype(np.float32)

    for c in range(NC):
        xeb = np.zeros((EPC, 16, P, CAP), np.float32)
        gwb = np.zeros((EPC, 8, P, 2048), np.float32)
        uwb = np.zeros((EPC, 8, P, 2048), np.float32)
        dwb = np.zeros((EPC, 8, P, 2048), np.float32)
        wrow = np.zeros((EPC, 1, CAP), np.float32)
        for i in range(EPC):
            e = EPC * c + i
            tok, tw = idx_lists[e], wt_lists[e]
            if len(tok) > CAP:
                spill.append((e, tok[CAP:], tw[CAP:]))
                tok, tw = tok[:CAP], tw[:CAP]
            n = len(tok)
            xeb[i, :, :, :n] = h2T[:, tok].reshape(16, P, n)
            wrow[i, 0, :n] = tw
            gwb[i] = _lhsT_prepack2(
                (w["gate_w"][e] * ln2[None, :]).T.astype(np.float32), 8)
            uwb[i] = _lhsT_prepack2(
                (w["up_w"][e] * ln2[None, :]).T.astype(np.float32), 8)
            dwb[i] = w["down_w"][e].T.reshape(8, P, H).astype(np.float32)
        rows = slice(256 * c, 256 * (c + 1))
        h2tp = np.ascontiguousarray(h2T[:, rows].reshape(16, P, 256)).astype(np.float32)
        per_core.append({
            "xe": xeb, "gw": gwb, "uw": uwb, "dw": dwb, "wrow": wrow,
            "h2t": h2tp, "sgw": sgw, "suw": suw, "sdw": sdw,
        })
    return per_core, idx_lists, wt_lists, spill


def _expert_np(h2ln, idx, wt, w, e):
    """Numpy fallback for capacity-overflow tokens."""
    xg = h2ln[idx]
    g = xg @ w["gate_w"][e].T
    u = xg @ w["up_w"][e].T
    a = (g / (1 + np.exp(-g))) * u
    return (a @ w["down_w"][e].T) * wt[:, None]


# ---------------------------------------------------------------- kernel
def kernel(**inputs):
    w = {k: np.asarray(v, dtype=np.float32) for k, v in inputs.items()}
    x = w["x"][0]                                           # [S, H]

    if "nc0" not in _cache:
        _cache["nc0"] = build_neff0()
    in0 = _neff0_inputs(x, w)
    res0 = _run(_cache["nc0"], in0, "neff0")
    qnT = np.concatenate([res0.results[c]["qn_out"] for c in range(NC)], axis=2)
    sig_all = np.concatenate([res0.results[c]["sig_out"] for c in range(NC)],
                             axis=2)                      # [2, 1, S]
    sig = np.ascontiguousarray(
        sig_all.reshape(2, NSTRIP, 1, 512).transpose(1, 0, 2, 3))

    if "nc1" not in _cache:
        _cache["nc1"] = build_neff1()
    nc1 = _cache["nc1"]
    in1 = _neff1_inputs(w, qnT, sig)
    res1 = _run(nc1, in1, "neff1")
    o_sum = np.zeros((S, H), np.float32)
    for c in range(NC):
        o_sum += res1.results[c]["o_part"]
    x2 = x + o_sum

    r2 = 1.0 / np.sqrt((x2 * x2).mean(-1, keepdims=True) + EPS)
    h2 = (x2 * r2).astype(np.float32)                       # rmsnorm w/o ln2
    h2ln = h2 * w["ln2_w"][None, :]
    order, wts = _route(h2ln, w)

    if "nc2" not in _cache:
        _cache["nc2"] = build_neff2()
    nc2 = _cache["nc2"]
    in2, idx_lists, wt_lists, spill = _neff2_inputs(h2, h2ln, w, order, wts)
    res2 = _run(nc2, in2, "neff2")

    out = x2.copy()
    for c in range(NC):
        r = res2.results[c]
        for i in range(EPC):
            e = EPC * c + i
            tok = idx_lists[e][:CAP]
            ye = r["yrT"][i].reshape(H, CAP).T              # [CAP, H]
            out[tok] += ye[:len(tok)]
        out[256 * c:256 * (c + 1)] += r["yshT"].reshape(H, 256).T
    for e, tok, tw in spill:
        out[tok] += _expert_np(h2ln, tok, tw, w, e)
    return out.reshape(1, S, H).astype(np.float32)



# revision 8
# speedup vs baseline: 1.2387x; 1.2387x over previous
"""nn_DecoderLayer (MLA attention + MoE routing) on 8 TRN2 NeuronCores.

Strategy:
  NEFF1 (attention): head-parallel — core c computes heads {2c, 2c+1}:
    replicated q_a/kv_a down-projections (feature-major, fp32r matmuls),
    per-head q_b/kv_b + RoPE (rotate-half folded into host-augmented
    weights), causal scoresT [k,q] layout, exp softmax without max
    subtraction (|scores| ~ 1.5), AV accumulate, partial o-projection.
    Host sums the 8 o-partials (expert-parallel style combine), adds
    residual, computes rmsnorm + router + top-4 routing in numpy.
  NEFF2 (MoE): expert-parallel — core c owns experts {2c, 2c+1}: gathered
    per-expert token batches (capacity CAP) through gate/up/silu/down with
    the combine weight folded into the activation; shared expert is
    token-parallel (core c handles tokens [256c, 256c+256)).
  Host scatters expert outputs back, adds shared + residual.

All matmuls fp32r (measured ~1.5e-4 rel err on HW at full PE rate).
"""
import math
import ml_dtypes
import numpy as np

import concourse.bacc as bacc
import concourse.mybir as mybir
import concourse.tile as tile
from concourse import bass_utils
from concourse.bass import ts
from concourse.masks import make_identity

# problem dims
S, H = 2048, 2048
NH, NOPE, ROPE, DV = 16, 128, 64, 128
DQK = NOPE + ROPE                  # 192
QR, KVR = 768, 512
E, KTOP, MI = 16, 4, 1024
SCALE = 2.5
EPS = 1e-6
ROPE_BASE = 10000.0

NC = 8                              # cores
HPC = NH // NC                      # heads/core = 2
EPC = E // NC                       # experts/core = 2
CAP = 512                           # per-expert token capacity (overflow -> host)
P = 128
NSTRIP = S // 512                   # 4 strips of 512 tokens

F32 = mybir.dt.float32
F32R = mybir.dt.float32r
BF16 = mybir.dt.bfloat16

Exp = mybir.ActivationFunctionType.Exp
Sqrt = mybir.ActivationFunctionType.Sqrt
Square = mybir.ActivationFunctionType.Square
Silu = mybir.ActivationFunctionType.Silu
Identity = mybir.ActivationFunctionType.Identity

_cache = {}

# profiling hooks (test.py sets TRACE=True; harness leaves it False)
TRACE = False
PROF = {}


def _run(nc, in_maps, name):
    if TRACE:
        res = bass_utils.run_bass_kernel_spmd(
            nc, in_maps, core_ids=list(range(NC)), trace=True, trace_cores=[0])
        it = res.instructions_and_trace or (None, None)
        PROF[name] = {"exec_time_ns": res.exec_time_ns, "trace": it[1],
                      "insts": it[0]}
        return res
    return bass_utils.run_bass_kernel_spmd(nc, in_maps,
                                           core_ids=list(range(NC)),
                                           trace=False)


# ---------------------------------------------------------------- NEFF 0
def build_neff0():
    """Token-sharded q_a/kv_a down-projections: core c handles tokens
    [256c, 256c+256). Outputs raw feature-major qn/kvn (bf16) + per-token
    sum-of-squares rows (host finishes the rmsnorm sigma)."""
    nc = bacc.Bacc("TRN2", num_devices=NC, debug=False)
    def inp(name, shape):
        return nc.dram_tensor(name, list(shape), BF16, kind="ExternalInput").ap()

    xTs = inp("xTs", (P, 16, 256))                 # my tokens, feature-major
    qaw = inp("qaw", (P, 6, 16, P))                # lhsT: [p, m_tile, h_chunk, j]
    kvaw = inp("kvaw", (P, 4, 16, P))
    qn_out = nc.dram_tensor("qn_out", [10, P, 256], BF16,
                            kind="ExternalOutput").ap()
    msq_out = nc.dram_tensor("msq_out", [2, 1, 256], F32,
                             kind="ExternalOutput").ap()

    with tile.TileContext(nc) as tc:
        with tc.tile_pool(name="p0", bufs=1) as p0, \
             tc.tile_pool(name="p02", bufs=2) as p02, \
             tc.tile_pool(name="ps0", bufs=2, space="PSUM") as ps0, \
             tc.tile_pool(name="ps0r", bufs=1, space="PSUM") as ps0r:
            ones_f = p0.tile([P, 1], F32)
            nc.vector.memset(ones_f, 1.0)
            ones_r = p0.tile([P, 1], F32R)
            nc.scalar.copy(ones_r, ones_f)
            qaw_sb = p0.tile([P, 6, 16, P], BF16)
            nc.sync.dma_start(out=qaw_sb, in_=qaw)
            kvaw_sb = p0.tile([P, 4, 16, P], BF16)
            nc.sync.dma_start(out=kvaw_sb, in_=kvaw)
            xs = p0.tile([P, 16, 256], BF16)
            nc.sync.dma_start(out=xs, in_=xTs)

            for kind in range(2):
                mt = 6 if kind == 0 else 4
                wsb = qaw_sb if kind == 0 else kvaw_sb
                off = 0 if kind == 0 else 6
                msq_ps = ps0r.tile([1, 256], F32, tag="row")
                for m in range(mt):
                    mm_ps = ps0.tile([P, 256], F32, tag="mm")
                    for c in range(16):
                        nc.tensor.matmul(mm_ps, wsb[:, m, c, :], xs[:, c, :],
                                         start=(c == 0), stop=(c == 15))
                    raw = p02.tile([P, 256], BF16, tag="raw")
                    nc.scalar.copy(raw, mm_ps)
                    nc.sync.dma_start(out=qn_out[off + m], in_=raw)
                    sq = p02.tile([P, 256], F32R, tag="sq")
                    nc.scalar.activation(out=sq, in_=mm_ps, func=Square)
                    nc.tensor.matmul(msq_ps, ones_r, sq,
                                     start=(m == 0), stop=(m == mt - 1))
                msq_row = p02.tile([1, 256], F32, tag="msqr")
                nc.scalar.copy(msq_row, msq_ps)
                nc.sync.dma_start(out=msq_out[kind], in_=msq_row)
    nc.compile()
    return nc


# ---------------------------------------------------------------- NEFF 1
def build_neff1():
    nc = bacc.Bacc("TRN2", num_devices=NC, debug=False)
    def inp(name, shape):
        return nc.dram_tensor(name, list(shape), F32, kind="ExternalInput").ap()

    qnT_i = inp("qnT", (10, P, S))                 # feature-major qn/kvn (unscaled)
    sig_i = inp("sig", (NSTRIP, 2, 1, 512))        # sigma rows
    qbw = inp("qbw", (P, 6, 512))                  # lhsT: [p, qr_chunk, m] (2 heads x 256)
    kvbw = inp("kvbw", (P, 4, 768))                # lhsT: [p, kvr_chunk, m] (2 heads x 384)
    ow = inp("ow", (P, HPC, H))                    # rhs: [p, c_chunk, h]
    cosT = inp("cosT", (ROPE, S))
    sinT = inp("sinT", (ROPE, S))
    masks = inp("masks", (P, 4, 512))              # tri01 keep-mask per diag offset
    o_part = nc.dram_tensor("o_part", [S, H], F32, kind="ExternalOutput").ap()

    with tile.TileContext(nc) as tc:
        with tc.tile_pool(name="const", bufs=1) as cpool, \
             tc.tile_pool(name="dram", bufs=1, space="DRAM") as dr:

            ones_f = cpool.tile([P, 1], F32)
            nc.vector.memset(ones_f, 1.0)
            ones_r = cpool.tile([P, 1], F32R)
            nc.scalar.copy(ones_r, ones_f)
            eps_t = cpool.tile([P, 1], F32)
            nc.vector.memset(eps_t, EPS)
            ident = cpool.tile([P, P], F32)
            make_identity(nc, ident)

            # DRAM scratch
            rcp_d = dr.tile([NSTRIP, HPC, 1, 512], F32)     # softmax denom bounce

            # ---------------- Windows B + C under the resident pool
            with tc.tile_pool(name="res", bufs=1) as res:
              qf_n = res.tile([P, HPC, S], F32R)
              qf_r = res.tile([64, HPC, S], F32R)
              kf_n = res.tile([P, HPC, S], F32R)
              kf_r = res.tile([64, HPC, S], F32R)
              v_sb = res.tile([P, HPC, 16, DV], F32R)
              # ---------------- Window B: q_b / kv_b + RoPE -> resident qf/kf/v
              with tc.tile_pool(name="pb", bufs=1) as pb, \
                   tc.tile_pool(name="pb2", bufs=2) as pb2, \
                   tc.tile_pool(name="pb1", bufs=1) as pb1, \
                   tc.tile_pool(name="psB", bufs=1, space="PSUM") as psB:
                  qbw_sb = pb.tile([P, 6, 512], F32R)
                  nc.sync.dma_start(out=qbw_sb, in_=qbw.bitcast(F32R))
                  kvbw_sb = pb.tile([P, 4, 768], F32R)
                  nc.sync.dma_start(out=kvbw_sb, in_=kvbw.bitcast(F32R))
                  cos_sb = pb.tile([ROPE, S], F32)
                  nc.sync.dma_start(out=cos_sb, in_=cosT)
                  sin_sb = pb.tile([ROPE, S], F32)
                  nc.sync.dma_start(out=sin_sb, in_=sinT)

                  for s in range(NSTRIP):
                      sl = slice(512 * s, 512 * (s + 1))
                      qn_t = pb2.tile([P, 10, 512], F32R, tag="qnt")
                      nc.sync.dma_start(out=qn_t,
                                        in_=qnT_i[:, :, sl].rearrange(
                                            "c p j -> p c j").bitcast(F32R))
                      sbcs = []
                      for kind in range(2):
                          sbc = pb1.tile([P, 512], F32, tag=f"sbc{kind}",
                                         name=f"sbc{kind}")
                          nc.sync.dma_start(out=sbc,
                                            in_=sig_i[s, kind].broadcast_to((P, 512)))
                          sbcs.append(sbc)
                      qn = [qn_t[:, c, :] for c in range(6)]
                      kvn = [qn_t[:, 6 + c, :] for c in range(4)]
                      for hi in range(HPC):
                          for side in range(2):  # 0: q, 1: k/v
                              if side == 0:
                                  wsb, chunks, base, nmt = qbw_sb, qn, 256 * hi, 6
                                  dn, rr, sbc = qf_n, qf_r, sbcs[0]
                              else:
                                  wsb, chunks, base, nmt = kvbw_sb, kvn, 384 * hi, 4
                                  dn, rr, sbc = kf_n, kf_r, sbcs[1]
                              ps_n = psB.tile([P, 512], F32, tag="bn")
                              ps_ro = psB.tile([64, 512], F32, tag="bro")
                              ps_rt = psB.tile([64, 512], F32, tag="brt")
                              for c in range(nmt):
                                  st, sp = (c == 0), (c == nmt - 1)
                                  nc.tensor.matmul(ps_n, wsb[:, c, base:base + 128],
                                                   chunks[c], start=st, stop=sp)
                                  nc.tensor.matmul(ps_ro,
                                                   wsb[:, c, base + 128:base + 192],
                                                   chunks[c], start=st, stop=sp)
                                  nc.tensor.matmul(ps_rt,
                                                   wsb[:, c, base + 192:base + 256],
                                                   chunks[c], start=st, stop=sp)
                              nc.vector.tensor_mul(dn[:, hi, sl], ps_n, sbc)
                              t1 = pb1.tile([64, 512], F32, tag="t1")
                              nc.vector.tensor_mul(t1, ps_ro, cos_sb[:, sl])
                              t2 = pb1.tile([64, 512], F32, tag="t2")
                              nc.vector.tensor_mul(t2, ps_rt, sin_sb[:, sl])
                              nc.vector.tensor_add(t1, t1, t2)
                              nc.vector.tensor_mul(rr[:, hi, sl], t1, sbc[0:64, :])
                              if side == 1:
                                  # v feature-major + sigma, then PE transpose
                                  ps_v = psB.tile([P, 512], F32, tag="bv")
                                  for c in range(4):
                                      nc.tensor.matmul(
                                          ps_v, kvbw_sb[:, c, base + 256:base + 384],
                                          chunks[c], start=(c == 0), stop=(c == 3))
                                  v_fm = pb1.tile([P, 512], F32, tag="vfm")
                                  nc.vector.tensor_mul(v_fm, ps_v, sbc)
                                  for t in range(4):
                                      ps_t = psB.tile([P, P], F32, tag="bt")
                                      nc.tensor.transpose(ps_t, v_fm[:, ts(t, P)],
                                                          ident)
                                      nc.scalar.copy(v_sb[:, hi, 4 * s + t, :], ps_t)

              # ---------------- Window C: attention + o-projection
              with tc.tile_pool(name="pc", bufs=1) as pc, \
                   tc.tile_pool(name="pc2", bufs=2) as pc2, \
                   tc.tile_pool(name="pc3", bufs=3) as pc3, \
                   tc.tile_pool(name="psC", bufs=2, space="PSUM") as psC, \
                   tc.tile_pool(name="psD", bufs=2, space="PSUM") as psD:
                  mask_sb = pc.tile([P, 4, 512], F32)
                  nc.sync.dma_start(out=mask_sb, in_=masks)
                  ow_sb = pc.tile([P, HPC, H], F32R)
                  nc.sync.dma_start(out=ow_sb, in_=ow.bitcast(F32R))

                  for s in range(NSTRIP):
                      sl = slice(512 * s, 512 * (s + 1))
                      ctx_sb = pc2.tile([P, HPC, 512], F32R, tag="ctx")
                      for hi in range(HPC):
                          ps_ctx = psD.tile([P, 512], F32, tag="ctx")
                          ps_den = psD.tile([1, 512], F32, tag="den")
                          nkc = 4 * s + 4
                          for kc in range(nkc):
                              ps_sc = psC.tile([P, 512], F32, tag="sc")
                              nc.tensor.matmul(ps_sc, kf_n[:, hi, ts(kc, P)],
                                               qf_n[:, hi, sl],
                                               start=True, stop=False)
                              nc.tensor.matmul(ps_sc, kf_r[:, hi, ts(kc, P)],
                                               qf_r[:, hi, sl],
                                               start=False, stop=True)
                              att = pc3.tile([P, 512], F32R, tag="att")
                              nc.scalar.activation(out=att, in_=ps_sc, func=Exp)
                              if kc >= 4 * s:
                                  nc.vector.tensor_mul(att, att.bitcast(F32),
                                                       mask_sb[:, kc - 4 * s, :])
                              nc.tensor.matmul(ps_den, ones_r, att,
                                               start=(kc == 0), stop=(kc == nkc - 1))
                              nc.tensor.matmul(ps_ctx, v_sb[:, hi, kc, :], att,
                                               start=(kc == 0), stop=(kc == nkc - 1))
                          den = pc2.tile([1, 512], F32, tag="den")
                          nc.scalar.copy(den, ps_den)
                          rcp = pc2.tile([1, 512], F32, tag="rcp")
                          nc.vector.reciprocal(rcp, den)
                          nc.sync.dma_start(out=rcp_d[s, hi], in_=rcp)
                          rcp_bc = pc2.tile([P, 512], F32, tag="rbc")
                          nc.sync.dma_start(out=rcp_bc,
                                            in_=rcp_d[s, hi].broadcast_to((P, 512)))
                          nc.vector.tensor_mul(ctx_sb[:, hi, :], ps_ctx, rcp_bc)
                      # o-projection for this strip (token-major out)
                      for t in range(4):
                          o_sb = pc2.tile([P, H], F32, tag="osb")
                          for hs in range(4):
                              ps_o = psC.tile([P, 512], F32, tag="o")
                              for cc in range(HPC):
                                  nc.tensor.matmul(ps_o, ctx_sb[:, cc, ts(t, P)],
                                                   ow_sb[:, cc, ts(hs, 512)],
                                                   start=(cc == 0), stop=(cc == HPC - 1))
                              nc.scalar.copy(o_sb[:, ts(hs, 512)], ps_o)
                          nc.sync.dma_start(
                              out=o_part.rearrange("(T p) h -> p T h", p=P)[:, 4 * s + t, :],
                              in_=o_sb)
    nc.compile()
    return nc


# ---------------------------------------------------------------- NEFF 2
def build_neff2():
    """Expert-parallel MoE, all-bf16 weights/activations (fp32 PSUM accum).

    Per expert slot (2 + shared): 8 gate/up m-tiles of [P,2048]->CAP
    through silu*up*combine, then 16 down h-tiles streamed [P,8,P].
    CAP=512 -> single PSUM strip per tile.
    """
    nc = bacc.Bacc("TRN2", num_devices=NC, debug=False)
    def inp(name, shape):
        return nc.dram_tensor(name, list(shape), BF16, kind="ExternalInput").ap()

    xe = inp("xe", (EPC, 16, P, CAP))        # gathered expert tokens (h2ln)
    gw = inp("gw", (EPC, 8, P, 2048))        # gate lhsT prepack
    uw = inp("uw", (EPC, 8, P, 2048))
    dw = inp("dw", (EPC, 16, P, 8, P))       # down lhsT per h-tile
    wrow = inp("wrow", (EPC, 1, CAP))        # combine weights (row layout)
    h2t = inp("h2t", (16, P, 256))           # my 256 tokens, feature-major
    sgw = inp("sgw", (8, P, 2048))
    suw = inp("suw", (8, P, 2048))
    sdw = inp("sdw", (16, P, 8, P))
    yrT = nc.dram_tensor("yrT", [EPC, 16, P, CAP], BF16,
                         kind="ExternalOutput").ap()
    yshT = nc.dram_tensor("yshT", [16, P, 256], BF16,
                          kind="ExternalOutput").ap()

    with tile.TileContext(nc) as tc:
        with tc.tile_pool(name="px", bufs=2) as px, \
             tc.tile_pool(name="pw", bufs=3) as pw, \
             tc.tile_pool(name="pd", bufs=3) as pd, \
             tc.tile_pool(name="pact", bufs=2) as pact, \
             tc.tile_pool(name="py", bufs=3) as py, \
             tc.tile_pool(name="ps", bufs=2, space="PSUM") as ps, \
             tc.tile_pool(name="psy", bufs=2, space="PSUM") as psy:

            def expert_block(xe_sb, w_bc, gw_i, uw_i, dw_i, y_out, n):
                """One expert's gate/up/silu/down over n token slots."""
                act = pact.tile([P, 8, CAP], BF16, tag="act")
                for t in range(8):
                    g_w = pw.tile([P, 16, P], BF16, tag="gw")
                    nc.sync.dma_start(out=g_w, in_=gw_i[t].rearrange(
                        "p (hc j) -> p hc j", j=P))
                    u_w = pw.tile([P, 16, P], BF16, tag="uw")
                    nc.sync.dma_start(out=u_w, in_=uw_i[t].rearrange(
                        "p (hc j) -> p hc j", j=P))
                    ps_g = ps.tile([P, n], F32, tag="g")
                    ps_u = ps.tile([P, n], F32, tag="u")
                    for hc in range(16):
                        st, sp = (hc == 0), (hc == 15)
                        nc.tensor.matmul(ps_g, g_w[:, hc, :], xe_sb[:, hc, 0:n],
                                         start=st, stop=sp)
                        nc.tensor.matmul(ps_u, u_w[:, hc, :], xe_sb[:, hc, 0:n],
                                         start=st, stop=sp)
                    sil = pw.tile([P, n], F32, tag="sil")
                    nc.scalar.activation(out=sil, in_=ps_g, func=Silu)
                    if w_bc is not None:
                        nc.gpsimd.tensor_mul(sil, sil, w_bc[:, 0:n])
                    nc.vector.tensor_mul(act[:, t, 0:n], sil, ps_u)
                for ht in range(16):
                    d_w = pd.tile([P, 8, P], BF16, tag="dw")
                    nc.sync.dma_start(out=d_w, in_=dw_i[ht])
                    ps_y = psy.tile([P, n], F32, tag="y")
                    for mc in range(8):
                        nc.tensor.matmul(ps_y, d_w[:, mc, :], act[:, mc, 0:n],
                                         start=(mc == 0), stop=(mc == 7))
                    y_sb = py.tile([P, n], BF16, tag="ysb")
                    nc.scalar.copy(y_sb, ps_y)
                    nc.sync.dma_start(out=y_out[ht], in_=y_sb)

            for i in range(EPC):
                xe_sb = px.tile([P, 16, CAP], BF16, tag="xe")
                nc.sync.dma_start(out=xe_sb,
                                  in_=xe[i].rearrange("hc p t -> p hc t"))
                w_bc = px.tile([P, CAP], BF16, tag="wbc")
                nc.sync.dma_start(out=w_bc, in_=wrow[i].broadcast_to((P, CAP)))
                expert_block(xe_sb, w_bc, gw[i], uw[i], dw[i], yrT[i], CAP)

            # ---------------- shared expert (my 256 tokens)
            h2_sb = px.tile([P, 16, 256], BF16, tag="h2")
            nc.sync.dma_start(out=h2_sb, in_=h2t.rearrange("hc p t -> p hc t"))
            expert_block(h2_sb, None, sgw, suw, sdw, yshT, 256)
    nc.compile()
    return nc


# ---------------------------------------------------------------- host prep
def _rope_tables():
    inv = 1.0 / (ROPE_BASE ** (np.arange(0, ROPE, 2, dtype=np.float64) / ROPE))
    t = np.arange(S, dtype=np.float64)
    f = t[:, None] * inv[None, :]
    emb = np.concatenate([f, f], axis=-1)          # [S, 64]
    return (np.cos(emb).T.astype(np.float32).copy(),
            np.sin(emb).T.astype(np.float32).copy())


def _lhsT_prepack(wT, mtiles):
    """wT [K, M] -> [P, mtiles, K//P, P]: SBUF-image for resident lhsT tiles."""
    Kd, Md = wT.shape
    assert Md == mtiles * P and Kd % P == 0
    return np.ascontiguousarray(
        wT.reshape(Kd // P, P, mtiles, P).transpose(1, 2, 0, 3))


def _lhsT_prepack2(wT, mtiles):
    """wT [K, M] -> [mtiles, P, K]: per-m-tile contiguous DMA layout.

    Tile t, flattened [P, K] with per-partition layout (hc, j):
    A[t, p, 128*hc + j] = wT[128*hc + p, 128*t + j].
    """
    Kd, Md = wT.shape
    assert Md == mtiles * P and Kd % P == 0
    return np.ascontiguousarray(
        wT.reshape(Kd // P, P, mtiles, P).transpose(2, 1, 0, 3).reshape(
            mtiles, P, Kd))


def _neff0_inputs(x, w):
    ln1 = w["ln1_w"]
    xT = x.T.astype(np.float32)                                   # [H, S]
    WqT = (w["q_a_w"] * ln1[None, :]).T.astype(np.float32)        # [H, QR]
    WkvT = (w["kv_a_w"] * ln1[None, :]).T.astype(np.float32)      # [H, KVR]
    qaw = _lhsT_prepack(WqT, 6)
    kvaw = _lhsT_prepack(WkvT, 4)
    arow = ((x * x).sum(-1) / H + EPS).astype(np.float32)         # [S]
    per_core = []
    for c in range(NC):
        rows = slice(256 * c, 256 * (c + 1))
        xTs = np.ascontiguousarray(xT[:, rows].reshape(16, P, 256)
                                   .transpose(1, 0, 2))
        per_core.append({"xTs": xTs, "qaw": qaw, "kvaw": kvaw,
                         "arow": arow[rows].reshape(1, 256)})
    return per_core


def _neff1_inputs(w, qnT, sig):
    qb = (w["q_b_w"] * w["q_a_ln"][None, :]).astype(np.float32)   # [NH*DQK, QR]
    kvb = (w["kv_b_w"] * w["kv_a_ln"][None, :]).astype(np.float32)  # [NH*320, KVR]
    sc = 1.0 / math.sqrt(DQK)
    cosT, sinT = _rope_tables()

    masks = np.zeros((P, 4, 512), np.float32)
    pp, jj = np.meshgrid(np.arange(P), np.arange(512), indexing="ij")
    for cl in range(4):
        masks[:, cl, :] = (P * cl + pp <= jj).astype(np.float32)

    per_core = []
    for c in range(NC):
        heads = [HPC * c + i for i in range(HPC)]
        # q_b augmented: per head rows [nope 128 | rope 64 | rot 64], scaled by sc
        qrows = []
        for h in heads:
            blk = qb[h * DQK:(h + 1) * DQK] * sc                   # [192, QR]
            nope, rope = blk[:NOPE], blk[NOPE:]
            rot = np.concatenate([-rope[32:], rope[:32]], axis=0)
            qrows.append(np.concatenate([nope, rope, rot], axis=0))  # [256, QR]
        qaug = np.concatenate(qrows, axis=0)                       # [512, QR]
        qbw = np.ascontiguousarray(qaug.T.reshape(6, P, 512).transpose(1, 0, 2))

        kvrows = []
        for h in heads:
            blk = kvb[h * 320:(h + 1) * 320]                       # [320, KVR]
            kn, kr, vv = blk[:NOPE], blk[NOPE:DQK], blk[DQK:]
            krot = np.concatenate([-kr[32:], kr[:32]], axis=0)
            kvrows.append(np.concatenate([kn, kr, krot, vv], axis=0))  # [384, KVR]
        kvaug = np.concatenate(kvrows, axis=0)                     # [768, KVR]
        kvbw = np.ascontiguousarray(kvaug.T.reshape(4, P, 768).transpose(1, 0, 2))

        ocols = np.concatenate([w["o_w"][:, h * DV:(h + 1) * DV] for h in heads],
                               axis=1)                             # [H, 256]
        owp = np.ascontiguousarray(ocols.T.reshape(HPC, P, H).transpose(1, 0, 2))

        per_core.append({
            "qnT": qnT, "sig": sig,
            "qbw": qbw.astype(np.float32), "kvbw": kvbw.astype(np.float32),
            "ow": owp.astype(np.float32), "cosT": cosT, "sinT": sinT,
            "masks": masks,
        })
    return per_core


def _route(h2ln, w):
    """Top-4 routing in numpy (fp32, matches jax semantics for these gaps)."""
    logits = h2ln @ w["router_w"].T.astype(np.float32) + w["router_b"][None, :]
    probs = 1.0 / (1.0 + np.exp(-logits))
    order = np.argsort(-probs, axis=-1, kind="stable")[:, :KTOP]
    topv = np.take_along_axis(probs, order, axis=-1)
    wts = topv / (topv.sum(-1, keepdims=True) + 1e-9) * SCALE
    return order, wts


def _down_prepack(dwT):
    """dwT [M, H] -> [16, P, 8, P]: per-h-tile lhsT chunks
    out[ht, p, mc, j] = dwT[mc*128+p, ht*128+j]."""
    return np.ascontiguousarray(
        dwT.reshape(8, P, 16, P).transpose(2, 1, 0, 3))


def _neff2_inputs(h2ln, w, order, wts):
    """Expert batches gathered from h2ln (ln2 pre-applied); bf16 payloads."""
    bf = ml_dtypes.bfloat16
    idx_lists, wt_lists = [], []
    for e in range(E):
        tok, kk = np.where(order == e)
        idx_lists.append(tok)
        wt_lists.append(wts[tok, kk])

    h2T = np.ascontiguousarray(h2ln.T).astype(bf)           # [H, S]
    per_core = []
    spill = []                                              # (expert, tok, wt) overflow
    sgw = _lhsT_prepack2(w["sg_w"].T.astype(np.float32), 8).astype(bf)
    suw = _lhsT_prepack2(w["su_w"].T.astype(np.float32), 8).astype(bf)
    sdw = _down_prepack(w["sd_w"].T).astype(bf)

    for c in range(NC):
        xeb = np.zeros((EPC, 16, P, CAP), bf)
        gwb = np.zeros((EPC, 8, P, 2048), bf)
        uwb = np.zeros((EPC, 8, P, 2048), bf)
        dwb = np.zeros((EPC, 16, P, 8, P), bf)
        wrow = np.zeros((EPC, 1, CAP), bf)
        for i in range(EPC):
            e = EPC * c + i
            tok, tw = idx_lists[e], wt_lists[e]
            if len(tok) > CAP:
                spill.append((e, tok[CAP:], tw[CAP:]))
                tok, tw = tok[:CAP], tw[:CAP]
            n = len(tok)
            xeb[i, :, :, :n] = h2T[:, tok].reshape(16, P, n)
            wrow[i, 0, :n] = tw.astype(bf)
            gwb[i] = _lhsT_prepack2(w["gate_w"][e].T.astype(np.float32), 8)
            uwb[i] = _lhsT_prepack2(w["up_w"][e].T.astype(np.float32), 8)
            dwb[i] = _down_prepack(w["down_w"][e].T)
        rows = slice(256 * c, 256 * (c + 1))
        h2tp = np.ascontiguousarray(h2T[:, rows].reshape(16, P, 256))
        per_core.append({
            "xe": xeb, "gw": gwb, "uw": uwb, "dw": dwb, "wrow": wrow,
            "h2t": h2tp, "sgw": sgw, "suw": suw, "sdw": sdw,
        })
    return per_core, idx_lists, wt_lists, spill


def _expert_np(h2ln, idx, wt, w, e):
    """Numpy fallback for capacity-overflow tokens."""
    xg = h2ln[idx]
    g = xg @ w["gate_w"][e].T
    u = xg @ w["up_w"][e].T
    a = (g / (1 + np.exp(-g))) * u
    return (a @ w["down_w"][e].T) * wt[:, None]


# ---------------------------------------------------------------- kernel
def kernel(**inputs):
    w = {k: np.asarray(v, dtype=np.float32) for k, v in inputs.items()}
    x = w["x"][0]                                           # [S, H]

    if "nc0" not in _cache:
        _cache["nc0"] = build_neff0()
    in0 = _neff0_inputs(x, w)
    res0 = _run(_cache["nc0"], in0, "neff0")
    qnT = np.concatenate([res0.results[c]["qn_out"] for c in range(NC)], axis=2)
    sig_all = np.concatenate([res0.results[c]["sig_out"] for c in range(NC)],
                             axis=2)                      # [2, 1, S]
    sig = np.ascontiguousarray(
        sig_all.reshape(2, NSTRIP, 1, 512).transpose(1, 0, 2, 3))

    if "nc1" not in _cache:
        _cache["nc1"] = build_neff1()
    nc1 = _cache["nc1"]
    in1 = _neff1_inputs(w, qnT, sig)
    res1 = _run(nc1, in1, "neff1")
    o_sum = np.zeros((S, H), np.float32)
    for c in range(NC):
        o_sum += res1.results[c]["o_part"]
    x2 = x + o_sum

    r2 = 1.0 / np.sqrt((x2 * x2).mean(-1, keepdims=True) + EPS)
    h2 = (x2 * r2).astype(np.float32)                       # rmsnorm w/o ln2
    h2ln = h2 * w["ln2_w"][None, :]
    order, wts = _route(h2ln, w)

    if "nc2" not in _cache:
        _cache["nc2"] = build_neff2()
    nc2 = _cache["nc2"]
    in2, idx_lists, wt_lists, spill = _neff2_inputs(h2ln, w, order, wts)
    res2 = _run(nc2, in2, "neff2")

    out = x2.copy()
    for c in range(NC):
        r = res2.results[c]
        for i in range(EPC):
            e = EPC * c + i
            tok = idx_lists[e][:CAP]
            ye = r["yrT"][i].reshape(H, CAP).T.astype(np.float32)  # [CAP, H]
            out[tok] += ye[:len(tok)]
        out[256 * c:256 * (c + 1)] += r["yshT"].reshape(H, 256).T.astype(
            np.float32)
    for e, tok, tw in spill:
        out[tok] += _expert_np(h2ln, tok, tw, w, e)
    return out.reshape(1, S, H).astype(np.float32)



# revision 15
# speedup vs baseline: 1.4012x; 1.1311x over previous
"""nn_DecoderLayer (MLA attention + MoE routing) on 8 TRN2 NeuronCores.

Strategy:
  NEFF1 (attention): head-parallel — core c computes heads {2c, 2c+1}:
    replicated q_a/kv_a down-projections (feature-major, fp32r matmuls),
    per-head q_b/kv_b + RoPE (rotate-half folded into host-augmented
    weights), causal scoresT [k,q] layout, exp softmax without max
    subtraction (|scores| ~ 1.5), AV accumulate, partial o-projection.
    Host sums the 8 o-partials (expert-parallel style combine), adds
    residual, computes rmsnorm + router + top-4 routing in numpy.
  NEFF2 (MoE): expert-parallel — core c owns experts {2c, 2c+1}: gathered
    per-expert token batches (capacity CAP) through gate/up/silu/down with
    the combine weight folded into the activation; shared expert is
    token-parallel (core c handles tokens [256c, 256c+256)).
  Host scatters expert outputs back, adds shared + residual.

All matmuls fp32r (measured ~1.5e-4 rel err on HW at full PE rate).
"""
import math
import ml_dtypes
import numpy as np

import concourse.bacc as bacc
import concourse.mybir as mybir
import concourse.tile as tile
from concourse import bass_utils
from concourse.bass import ts
from concourse.masks import make_identity

# problem dims
S, H = 2048, 2048
NH, NOPE, ROPE, DV = 16, 128, 64, 128
DQK = NOPE + ROPE                  # 192
QR, KVR = 768, 512
E, KTOP, MI = 16, 4, 1024
SCALE = 2.5
EPS = 1e-6
ROPE_BASE = 10000.0

NC = 8                              # cores
HPC = NH // NC                      # heads/core = 2
EPC = E // NC                       # experts/core = 2
CAP = 512                           # per-expert token capacity (overflow -> host)
P = 128
NSTRIP = S // 512                   # 4 strips of 512 tokens

F32 = mybir.dt.float32
F32R = mybir.dt.float32r
BF16 = mybir.dt.bfloat16

Exp = mybir.ActivationFunctionType.Exp
Sqrt = mybir.ActivationFunctionType.Sqrt
Square = mybir.ActivationFunctionType.Square
Silu = mybir.ActivationFunctionType.Silu
Identity = mybir.ActivationFunctionType.Identity

_cache = {}

# profiling hooks (test.py sets TRACE=True; harness leaves it False)
TRACE = False
PROF = {}


def _run(nc, in_maps, name):
    if TRACE:
        res = bass_utils.run_bass_kernel_spmd(
            nc, in_maps, core_ids=list(range(NC)), trace=True, trace_cores=[0])
        it = res.instructions_and_trace or (None, None)
        PROF[name] = {"exec_time_ns": res.exec_time_ns, "trace": it[1],
                      "insts": it[0]}
        return res
    return bass_utils.run_bass_kernel_spmd(nc, in_maps,
                                           core_ids=list(range(NC)),
                                           trace=False)


# ---------------------------------------------------------------- NEFF 0
def build_neff0():
    """Token-sharded q_a/kv_a down-projections: core c handles tokens
    [256c, 256c+256). Outputs raw feature-major qn/kvn (bf16) + per-token
    sum-of-squares rows (host finishes the rmsnorm sigma)."""
    nc = bacc.Bacc("TRN2", num_devices=NC, debug=False)
    def inp(name, shape):
        return nc.dram_tensor(name, list(shape), BF16, kind="ExternalInput").ap()

    xTs = inp("xTs", (P, 16, 256))                 # my tokens, feature-major
    qaw = inp("qaw", (P, 6, 16, P))                # lhsT: [p, m_tile, h_chunk, j]
    kvaw = inp("kvaw", (P, 4, 16, P))
    qn_out = nc.dram_tensor("qn_out", [10, P, 256], BF16,
                            kind="ExternalOutput").ap()
    msq_out = nc.dram_tensor("msq_out", [2, 1, 256], F32,
                             kind="ExternalOutput").ap()

    with tile.TileContext(nc) as tc:
        with tc.tile_pool(name="p0", bufs=1) as p0, \
             tc.tile_pool(name="p02", bufs=2) as p02, \
             tc.tile_pool(name="ps0", bufs=2, space="PSUM") as ps0, \
             tc.tile_pool(name="ps0r", bufs=1, space="PSUM") as ps0r:
            ones_f = p0.tile([P, 1], F32)
            nc.vector.memset(ones_f, 1.0)
            ones_r = p0.tile([P, 1], F32R)
            nc.scalar.copy(ones_r, ones_f)
            qaw_sb = p0.tile([P, 6, 16, P], BF16)
            nc.sync.dma_start(out=qaw_sb, in_=qaw)
            kvaw_sb = p0.tile([P, 4, 16, P], BF16)
            nc.sync.dma_start(out=kvaw_sb, in_=kvaw)
            xs = p0.tile([P, 16, 256], BF16)
            nc.sync.dma_start(out=xs, in_=xTs)

            for kind in range(2):
                mt = 6 if kind == 0 else 4
                wsb = qaw_sb if kind == 0 else kvaw_sb
                off = 0 if kind == 0 else 6
                msq_ps = ps0r.tile([1, 256], F32, tag="row")
                for m in range(mt):
                    mm_ps = ps0.tile([P, 256], F32, tag="mm")
                    for c in range(16):
                        nc.tensor.matmul(mm_ps, wsb[:, m, c, :], xs[:, c, :],
                                         start=(c == 0), stop=(c == 15))
                    raw = p02.tile([P, 256], BF16, tag="raw")
                    nc.scalar.copy(raw, mm_ps)
                    nc.sync.dma_start(out=qn_out[off + m], in_=raw)
                    sq = p02.tile([P, 256], F32R, tag="sq")
                    nc.scalar.activation(out=sq, in_=mm_ps, func=Square)
                    nc.tensor.matmul(msq_ps, ones_r, sq,
                                     start=(m == 0), stop=(m == mt - 1))
                msq_row = p02.tile([1, 256], F32, tag="msqr")
                nc.scalar.copy(msq_row, msq_ps)
                nc.sync.dma_start(out=msq_out[kind], in_=msq_row)
    nc.compile()
    return nc


# ---------------------------------------------------------------- NEFF 1
def build_neff1():
    """Head-parallel attention, bf16 operands with fp32 PSUM.

    qnT arrives pre-scaled by the rmsnorm sigmas (host-folded), so window B
    is pure matmul + RoPE combine. Augmented weight layout per head:
    q: [nope 128 | rope 64; rot 64], kv: [kn 128 | kr 64; krot 64 | v 128]
    -> every matmul runs with M=128.
    """
    nc = bacc.Bacc("TRN2", num_devices=NC, debug=False)
    def inp(name, shape, dt=BF16):
        return nc.dram_tensor(name, list(shape), dt, kind="ExternalInput").ap()

    qnT_i = inp("qnT", (10, P, S))                 # feature-major qn/kvn (scaled)
    qbw = inp("qbw", (P, 6, 512))                  # lhsT: [p, qr_chunk, m]
    kvbw = inp("kvbw", (P, 4, 768))                # lhsT: [p, kvr_chunk, m]
    ow = inp("ow", (P, HPC, H))                    # rhs: [p, c_chunk, h]
    cosT = inp("cosT", (ROPE, S), F32)
    sinT = inp("sinT", (ROPE, S), F32)
    masks = inp("masks", (P, 4, 512))              # tri01 keep-mask per diag offset
    o_part = nc.dram_tensor("o_part", [S, H], BF16, kind="ExternalOutput").ap()

    with tile.TileContext(nc) as tc:
        with tc.tile_pool(name="const", bufs=1) as cpool, \
             tc.tile_pool(name="dram", bufs=1, space="DRAM") as dr:

            ones_f = cpool.tile([P, 1], F32)
            nc.vector.memset(ones_f, 1.0)
            ones_b = cpool.tile([P, 1], BF16)
            nc.scalar.copy(ones_b, ones_f)
            ident = cpool.tile([P, P], BF16)
            make_identity(nc, ident)

            # DRAM scratch
            rcp_d = dr.tile([NSTRIP, HPC, 1, 512], F32)     # softmax denom bounce

            # ---------------- Windows B + C under the resident pool
            with tc.tile_pool(name="res", bufs=1) as res:
              qf_n = res.tile([P, HPC, S], BF16)
              qf_r = res.tile([64, HPC, S], BF16)
              kf_n = res.tile([P, HPC, S], BF16)
              kf_r = res.tile([64, HPC, S], BF16)
              v_sb = res.tile([P, HPC, 16, DV], BF16)
              # ---------------- Window B: q_b / kv_b + RoPE -> resident qf/kf/v
              with tc.tile_pool(name="pb", bufs=1) as pb, \
                   tc.tile_pool(name="pb2", bufs=2) as pb2, \
                   tc.tile_pool(name="pb1", bufs=2) as pb1, \
                   tc.tile_pool(name="psB", bufs=2, space="PSUM") as psB:
                  qbw_sb = pb.tile([P, 6, 512], BF16)
                  nc.sync.dma_start(out=qbw_sb, in_=qbw)
                  kvbw_sb = pb.tile([P, 4, 768], BF16)
                  nc.sync.dma_start(out=kvbw_sb, in_=kvbw)
                  cos_sb = pb.tile([ROPE, S], F32)
                  nc.sync.dma_start(out=cos_sb, in_=cosT)
                  sin_sb = pb.tile([ROPE, S], F32)
                  nc.sync.dma_start(out=sin_sb, in_=sinT)

                  for s in range(NSTRIP):
                      sl = slice(512 * s, 512 * (s + 1))
                      qn_t = pb2.tile([P, 10, 512], BF16, tag="qnt")
                      nc.sync.dma_start(out=qn_t,
                                        in_=qnT_i[:, :, sl].rearrange(
                                            "c p j -> p c j"))
                      qn = [qn_t[:, c, :] for c in range(6)]
                      kvn = [qn_t[:, 6 + c, :] for c in range(4)]
                      for hi in range(HPC):
                          for side in range(2):  # 0: q, 1: k/v
                              if side == 0:
                                  wsb, chunks, base, nmt = qbw_sb, qn, 256 * hi, 6
                                  dn, rr = qf_n, qf_r
                              else:
                                  wsb, chunks, base, nmt = kvbw_sb, kvn, 384 * hi, 4
                                  dn, rr = kf_n, kf_r
                              ps_n = psB.tile([P, 512], F32, tag="bn")
                              ps_ror = psB.tile([P, 512], F32, tag="bro")
                              for c in range(nmt):
                                  st, sp = (c == 0), (c == nmt - 1)
                                  nc.tensor.matmul(ps_n, wsb[:, c, base:base + 128],
                                                   chunks[c], start=st, stop=sp)
                                  nc.tensor.matmul(ps_ror,
                                                   wsb[:, c, base + 128:base + 256],
                                                   chunks[c], start=st, stop=sp)
                              nc.scalar.copy(dn[:, hi, sl], ps_n)
                              t1 = pb1.tile([64, 512], F32, tag="t1")
                              nc.vector.tensor_mul(t1, ps_ror[0:64, :],
                                                   cos_sb[:, sl])
                              t2 = pb1.tile([64, 512], F32, tag="t2")
                              nc.vector.tensor_mul(t2, ps_ror[64:128, :],
                                                   sin_sb[:, sl])
                              nc.vector.tensor_add(rr[:, hi, sl], t1, t2)
                              if side == 1:
                                  # v feature-major, then PE transpose
                                  ps_v = psB.tile([P, 512], F32, tag="bv")
                                  for c in range(4):
                                      nc.tensor.matmul(
                                          ps_v, kvbw_sb[:, c, base + 256:base + 384],
                                          chunks[c], start=(c == 0), stop=(c == 3))
                                  v_fm = pb1.tile([P, 512], BF16, tag="vfm")
                                  nc.scalar.copy(v_fm, ps_v)
                                  for t in range(4):
                                      ps_t = psB.tile([P, P], BF16, tag="bt")
                                      nc.tensor.transpose(ps_t, v_fm[:, ts(t, P)],
                                                          ident)
                                      nc.scalar.copy(v_sb[:, hi, 4 * s + t, :], ps_t)

              # ---------------- Window C: attention + o-projection
              with tc.tile_pool(name="pc", bufs=1) as pc, \
                   tc.tile_pool(name="pc2", bufs=2) as pc2, \
                   tc.tile_pool(name="pc3", bufs=3) as pc3, \
                   tc.tile_pool(name="psC", bufs=2, space="PSUM") as psC, \
                   tc.tile_pool(name="psD", bufs=2, space="PSUM") as psD:
                  mask_sb = pc.tile([P, 4, 512], BF16)
                  nc.sync.dma_start(out=mask_sb, in_=masks)
                  ow_sb = pc.tile([P, HPC, H], BF16)
                  nc.sync.dma_start(out=ow_sb, in_=ow)

                  for s in range(NSTRIP):
                      sl = slice(512 * s, 512 * (s + 1))
                      ctx_sb = pc2.tile([P, HPC, 512], BF16, tag="ctx")
                      for hi in range(HPC):
                          ps_ctx = psD.tile([P, 512], F32, tag="ctx")
                          ps_den = psD.tile([1, 512], F32, tag="den")
                          nkc = 4 * s + 4
                          for kc in range(nkc):
                              ps_sc = psC.tile([P, 512], F32, tag="sc")
                              nc.tensor.matmul(ps_sc, kf_n[:, hi, ts(kc, P)],
                                               qf_n[:, hi, sl],
                                               start=True, stop=False)
                              nc.tensor.matmul(ps_sc, kf_r[:, hi, ts(kc, P)],
                                               qf_r[:, hi, sl],
                                               start=False, stop=True)
                              att = pc3.tile([P, 512], BF16, tag="att")
                              nc.scalar.activation(out=att, in_=ps_sc, func=Exp)
                              if kc >= 4 * s:
                                  nc.vector.tensor_mul(att, att,
                                                       mask_sb[:, kc - 4 * s, :])
                              nc.tensor.matmul(ps_den, ones_b, att,
                                               start=(kc == 0), stop=(kc == nkc - 1))
                              nc.tensor.matmul(ps_ctx, v_sb[:, hi, kc, :], att,
                                               start=(kc == 0), stop=(kc == nkc - 1))
                          rcp = pc2.tile([1, 512], F32, tag="rcp")
                          nc.vector.reciprocal_approx_fast(out=rcp, in_=ps_den)
                          nc.sync.dma_start(out=rcp_d[s, hi], in_=rcp)
                          rcp_bc = pc2.tile([P, 512], F32, tag="rbc")
                          nc.sync.dma_start(out=rcp_bc,
                                            in_=rcp_d[s, hi].broadcast_to((P, 512)))
                          nc.vector.tensor_mul(ctx_sb[:, hi, :], ps_ctx, rcp_bc)
                      # o-projection for this strip (token-major out)
                      for t in range(4):
                          o_sb = pc2.tile([P, H], BF16, tag="osb")
                          for hs in range(4):
                              ps_o = psC.tile([P, 512], F32, tag="o")
                              for cc in range(HPC):
                                  nc.tensor.matmul(ps_o, ctx_sb[:, cc, ts(t, P)],
                                                   ow_sb[:, cc, ts(hs, 512)],
                                                   start=(cc == 0), stop=(cc == HPC - 1))
                              nc.vector.tensor_copy(o_sb[:, ts(hs, 512)], ps_o)
                          nc.sync.dma_start(
                              out=o_part.rearrange("(T p) h -> p T h", p=P)[:, 4 * s + t, :],
                              in_=o_sb)
    nc.compile()
    return nc


# ---------------------------------------------------------------- NEFF 2
def build_neff2():
    """Expert-parallel MoE, all-bf16 weights/activations (fp32 PSUM accum).

    Per expert slot (2 + shared): 8 gate/up m-tiles of [P,2048]->CAP
    through silu*up*combine, then 16 down h-tiles streamed [P,8,P].
    CAP=512 -> single PSUM strip per tile.
    """
    nc = bacc.Bacc("TRN2", num_devices=NC, debug=False)
    def inp(name, shape):
        return nc.dram_tensor(name, list(shape), BF16, kind="ExternalInput").ap()

    xe = inp("xe", (EPC, 16, P, CAP))        # gathered expert tokens (h2ln)
    gw = inp("gw", (EPC, 8, P, 2048))        # gate lhsT prepack
    uw = inp("uw", (EPC, 8, P, 2048))
    dw = inp("dw", (EPC, 16, P, 8, P))       # down lhsT per h-tile
    wrow = inp("wrow", (EPC, 1, CAP))        # combine weights (row layout)
    h2t = inp("h2t", (16, P, 256))           # my 256 tokens, feature-major
    sgw = inp("sgw", (8, P, 2048))
    suw = inp("suw", (8, P, 2048))
    sdw = inp("sdw", (16, P, 8, P))
    yrT = nc.dram_tensor("yrT", [EPC, 16, P, CAP], BF16,
                         kind="ExternalOutput").ap()
    yshT = nc.dram_tensor("yshT", [16, P, 256], BF16,
                          kind="ExternalOutput").ap()

    with tile.TileContext(nc) as tc:
        with tc.tile_pool(name="px", bufs=2) as px, \
             tc.tile_pool(name="pw", bufs=3) as pw, \
             tc.tile_pool(name="pd", bufs=3) as pd, \
             tc.tile_pool(name="pact", bufs=2) as pact, \
             tc.tile_pool(name="py", bufs=3) as py, \
             tc.tile_pool(name="ps", bufs=2, space="PSUM") as ps, \
             tc.tile_pool(name="psy", bufs=2, space="PSUM") as psy:

            def expert_block(xe_sb, w_bc, gw_i, uw_i, dw_i, y_out, n):
                """One expert's gate/up/silu/down over n token slots."""
                act = pact.tile([P, 8, CAP], BF16, tag="act")
                for t in range(8):
                    g_w = pw.tile([P, 16, P], BF16, tag="gw")
                    nc.sync.dma_start(out=g_w, in_=gw_i[t].rearrange(
                        "p (hc j) -> p hc j", j=P))
                    u_w = pw.tile([P, 16, P], BF16, tag="uw")
                    nc.sync.dma_start(out=u_w, in_=uw_i[t].rearrange(
                        "p (hc j) -> p hc j", j=P))
                    ps_g = ps.tile([P, n], F32, tag="g")
                    ps_u = ps.tile([P, n], F32, tag="u")
                    for hc in range(16):
                        st, sp = (hc == 0), (hc == 15)
                        nc.tensor.matmul(ps_g, g_w[:, hc, :], xe_sb[:, hc, 0:n],
                                         start=st, stop=sp)
                        nc.tensor.matmul(ps_u, u_w[:, hc, :], xe_sb[:, hc, 0:n],
                                         start=st, stop=sp)
                    sil = pw.tile([P, n], F32, tag="sil")
                    nc.scalar.activation(out=sil, in_=ps_g, func=Silu)
                    if w_bc is not None:
                        nc.gpsimd.tensor_mul(sil, sil, w_bc[:, 0:n])
                    nc.vector.tensor_mul(act[:, t, 0:n], sil, ps_u)
                for ht in range(16):
                    d_w = pd.tile([P, 8, P], BF16, tag="dw")
                    nc.sync.dma_start(out=d_w, in_=dw_i[ht])
                    ps_y = psy.tile([P, n], F32, tag="y")
                    for mc in range(8):
                        nc.tensor.matmul(ps_y, d_w[:, mc, :], act[:, mc, 0:n],
                                         start=(mc == 0), stop=(mc == 7))
                    y_sb = py.tile([P, n], BF16, tag="ysb")
                    nc.scalar.copy(y_sb, ps_y)
                    nc.sync.dma_start(out=y_out[ht], in_=y_sb)

            for i in range(EPC):
                xe_sb = px.tile([P, 16, CAP], BF16, tag="xe")
                nc.sync.dma_start(out=xe_sb,
                                  in_=xe[i].rearrange("hc p t -> p hc t"))
                w_bc = px.tile([P, CAP], BF16, tag="wbc")
                nc.sync.dma_start(out=w_bc, in_=wrow[i].broadcast_to((P, CAP)))
                expert_block(xe_sb, w_bc, gw[i], uw[i], dw[i], yrT[i], CAP)

            # ---------------- shared expert (my 256 tokens)
            h2_sb = px.tile([P, 16, 256], BF16, tag="h2")
            nc.sync.dma_start(out=h2_sb, in_=h2t.rearrange("hc p t -> p hc t"))
            expert_block(h2_sb, None, sgw, suw, sdw, yshT, 256)
    nc.compile()
    return nc


# ---------------------------------------------------------------- host prep
def _rope_tables():
    inv = 1.0 / (ROPE_BASE ** (np.arange(0, ROPE, 2, dtype=np.float64) / ROPE))
    t = np.arange(S, dtype=np.float64)
    f = t[:, None] * inv[None, :]
    emb = np.concatenate([f, f], axis=-1)          # [S, 64]
    return (np.cos(emb).T.astype(np.float32).copy(),
            np.sin(emb).T.astype(np.float32).copy())


def _lhsT_prepack(wT, mtiles):
    """wT [K, M] -> [P, mtiles, K//P, P]: SBUF-image for resident lhsT tiles."""
    Kd, Md = wT.shape
    assert Md == mtiles * P and Kd % P == 0
    return np.ascontiguousarray(
        wT.reshape(Kd // P, P, mtiles, P).transpose(1, 2, 0, 3))


def _lhsT_prepack2(wT, mtiles):
    """wT [K, M] -> [mtiles, P, K]: per-m-tile contiguous DMA layout.

    Tile t, flattened [P, K] with per-partition layout (hc, j):
    A[t, p, 128*hc + j] = wT[128*hc + p, 128*t + j].
    """
    Kd, Md = wT.shape
    assert Md == mtiles * P and Kd % P == 0
    return np.ascontiguousarray(
        wT.reshape(Kd // P, P, mtiles, P).transpose(2, 1, 0, 3).reshape(
            mtiles, P, Kd))


def _neff0_inputs(x, w):
    bf = ml_dtypes.bfloat16
    ln1 = w["ln1_w"]
    xT = x.T.astype(np.float32)                                   # [H, S]
    WqT = (w["q_a_w"] * ln1[None, :]).T.astype(np.float32)        # [H, QR]
    WkvT = (w["kv_a_w"] * ln1[None, :]).T.astype(np.float32)      # [H, KVR]
    qaw = _lhsT_prepack(WqT, 6).astype(bf)
    kvaw = _lhsT_prepack(WkvT, 4).astype(bf)
    per_core = []
    for c in range(NC):
        rows = slice(256 * c, 256 * (c + 1))
        xTs = np.ascontiguousarray(xT[:, rows].reshape(16, P, 256)
                                   .transpose(1, 0, 2)).astype(bf)
        per_core.append({"xTs": xTs, "qaw": qaw, "kvaw": kvaw})
    return per_core


def _neff1_inputs(w, qnT):
    bf = ml_dtypes.bfloat16
    qb = (w["q_b_w"] * w["q_a_ln"][None, :]).astype(np.float32)   # [NH*DQK, QR]
    kvb = (w["kv_b_w"] * w["kv_a_ln"][None, :]).astype(np.float32)  # [NH*320, KVR]
    sc = 1.0 / math.sqrt(DQK)
    cosT, sinT = _rope_tables()

    masks = np.zeros((P, 4, 512), np.float32)
    pp, jj = np.meshgrid(np.arange(P), np.arange(512), indexing="ij")
    for cl in range(4):
        masks[:, cl, :] = (P * cl + pp <= jj).astype(np.float32)
    masks = masks.astype(bf)

    per_core = []
    for c in range(NC):
        heads = [HPC * c + i for i in range(HPC)]
        # q_b augmented: per head rows [nope 128 | rope 64 | rot 64], scaled by sc
        qrows = []
        for h in heads:
            blk = qb[h * DQK:(h + 1) * DQK] * sc                   # [192, QR]
            nope, rope = blk[:NOPE], blk[NOPE:]
            rot = np.concatenate([-rope[32:], rope[:32]], axis=0)
            qrows.append(np.concatenate([nope, rope, rot], axis=0))  # [256, QR]
        qaug = np.concatenate(qrows, axis=0)                       # [512, QR]
        qbw = np.ascontiguousarray(qaug.T.reshape(6, P, 512).transpose(1, 0, 2))

        kvrows = []
        for h in heads:
            blk = kvb[h * 320:(h + 1) * 320]                       # [320, KVR]
            kn, kr, vv = blk[:NOPE], blk[NOPE:DQK], blk[DQK:]
            krot = np.concatenate([-kr[32:], kr[:32]], axis=0)
            kvrows.append(np.concatenate([kn, kr, krot, vv], axis=0))  # [384, KVR]
        kvaug = np.concatenate(kvrows, axis=0)                     # [768, KVR]
        kvbw = np.ascontiguousarray(kvaug.T.reshape(4, P, 768).transpose(1, 0, 2))

        ocols = np.concatenate([w["o_w"][:, h * DV:(h + 1) * DV] for h in heads],
                               axis=1)                             # [H, 256]
        owp = np.ascontiguousarray(ocols.T.reshape(HPC, P, H).transpose(1, 0, 2))

        per_core.append({
            "qnT": qnT,
            "qbw": qbw.astype(bf), "kvbw": kvbw.astype(bf),
            "ow": owp.astype(bf), "cosT": cosT, "sinT": sinT,
            "masks": masks,
        })
    return per_core


def _rmsn(v, eps=EPS):
    return v / np.sqrt((v * v).mean(-1, keepdims=True) + eps)


def _exact_probs(x, w, tb):
    """fp32 numpy router probs for tokens tb: exact attention rows ->
    exact x2 rows -> exact logits. Borderline-token routing must match the
    fp32 reference; the device's bf16 x2 is too coarse for near-ties."""
    h1 = _rmsn(x) * w["ln1_w"]
    kvl = _rmsn(h1 @ w["kv_a_w"].T) * w["kv_a_ln"]
    kvp = (kvl @ w["kv_b_w"].T).reshape(S, NH, DQK + DV)
    kn, kr, vv = kvp[..., :NOPE], kvp[..., NOPE:DQK], kvp[..., DQK:]
    inv = 1.0 / (ROPE_BASE ** (np.arange(0, ROPE, 2, dtype=np.float64) / ROPE))
    f = np.arange(S, dtype=np.float64)[:, None] * inv[None, :]
    emb = np.concatenate([f, f], axis=-1)
    cos = np.cos(emb).astype(np.float32)[:, None, :]
    sin = np.sin(emb).astype(np.float32)[:, None, :]
    rot = lambda z: np.concatenate([-z[..., 32:], z[..., :32]], axis=-1)
    kf = np.concatenate([kn, kr * cos + rot(kr) * sin], axis=-1)  # [S,NH,192]

    h1b = h1[tb]
    ql = _rmsn(h1b @ w["q_a_w"].T) * w["q_a_ln"]
    qp = (ql @ w["q_b_w"].T).reshape(len(tb), NH, DQK)
    qn_, qr_ = qp[..., :NOPE], qp[..., NOPE:]
    qr_ = qr_ * cos[tb] + rot(qr_) * sin[tb]
    qf = np.concatenate([qn_, qr_], axis=-1)                      # [B,NH,192]

    sc = np.einsum("bhd,khd->bhk", qf, kf) / math.sqrt(DQK)
    keymask = np.arange(S)[None, :] > tb[:, None]                 # [B,S]
    sc = sc - 1e9 * keymask[:, None, :]
    sc -= sc.max(-1, keepdims=True)
    att = np.exp(sc)
    att /= att.sum(-1, keepdims=True)
    ctx = np.einsum("bhk,khd->bhd", att, vv).reshape(len(tb), NH * DV)
    x2r = x[tb] + ctx @ w["o_w"].T
    h2r = _rmsn(x2r) * w["ln2_w"]
    logits = h2r @ w["router_w"].T + w["router_b"][None, :]
    return 1.0 / (1.0 + np.exp(-logits))


def _route(h2ln, w, x):
    """Top-4 routing in numpy; borderline tokens (4th vs 5th prob near-tie)
    get their probs recomputed exactly so the discrete choice matches the
    fp32 reference."""
    logits = h2ln @ w["router_w"].T.astype(np.float32) + w["router_b"][None, :]
    probs = 1.0 / (1.0 + np.exp(-logits))
    sp = np.sort(probs, axis=-1)
    gap = sp[:, -KTOP] - sp[:, -KTOP - 1]
    tb = np.where(gap < 0.01)[0]
    if len(tb):
        probs[tb] = _exact_probs(x, w, tb)
    order = np.argsort(-probs, axis=-1, kind="stable")[:, :KTOP]
    topv = np.take_along_axis(probs, order, axis=-1)
    wts = topv / (topv.sum(-1, keepdims=True) + 1e-9) * SCALE
    return order, wts


def _down_prepack(dwT):
    """dwT [M, H] -> [16, P, 8, P]: per-h-tile lhsT chunks
    out[ht, p, mc, j] = dwT[mc*128+p, ht*128+j]."""
    return np.ascontiguousarray(
        dwT.reshape(8, P, 16, P).transpose(2, 1, 0, 3))


def _neff2_inputs(h2ln, w, order, wts):
    """Expert batches gathered from h2ln (ln2 pre-applied); bf16 payloads."""
    bf = ml_dtypes.bfloat16
    idx_lists, wt_lists = [], []
    for e in range(E):
        tok, kk = np.where(order == e)
        idx_lists.append(tok)
        wt_lists.append(wts[tok, kk])

    h2T = np.ascontiguousarray(h2ln.T).astype(bf)           # [H, S]
    per_core = []
    spill = []                                              # (expert, tok, wt) overflow
    sgw = _lhsT_prepack2(w["sg_w"].T.astype(np.float32), 8).astype(bf)
    suw = _lhsT_prepack2(w["su_w"].T.astype(np.float32), 8).astype(bf)
    sdw = _down_prepack(w["sd_w"].T).astype(bf)

    for c in range(NC):
        xeb = np.zeros((EPC, 16, P, CAP), bf)
        gwb = np.zeros((EPC, 8, P, 2048), bf)
        uwb = np.zeros((EPC, 8, P, 2048), bf)
        dwb = np.zeros((EPC, 16, P, 8, P), bf)
        wrow = np.zeros((EPC, 1, CAP), bf)
        for i in range(EPC):
            e = EPC * c + i
            tok, tw = idx_lists[e], wt_lists[e]
            if len(tok) > CAP:
                spill.append((e, tok[CAP:], tw[CAP:]))
                tok, tw = tok[:CAP], tw[:CAP]
            n = len(tok)
            xeb[i, :, :, :n] = h2T[:, tok].reshape(16, P, n)
            wrow[i, 0, :n] = tw.astype(bf)
            gwb[i] = _lhsT_prepack2(w["gate_w"][e].T.astype(np.float32), 8)
            uwb[i] = _lhsT_prepack2(w["up_w"][e].T.astype(np.float32), 8)
            dwb[i] = _down_prepack(w["down_w"][e].T)
        rows = slice(256 * c, 256 * (c + 1))
        h2tp = np.ascontiguousarray(h2T[:, rows].reshape(16, P, 256))
        per_core.append({
            "xe": xeb, "gw": gwb, "uw": uwb, "dw": dwb, "wrow": wrow,
            "h2t": h2tp, "sgw": sgw, "suw": suw, "sdw": sdw,
        })
    return per_core, idx_lists, wt_lists, spill


def _expert_np(h2ln, idx, wt, w, e):
    """Numpy fallback for capacity-overflow tokens."""
    xg = h2ln[idx]
    g = xg @ w["gate_w"][e].T
    u = xg @ w["up_w"][e].T
    a = (g / (1 + np.exp(-g))) * u
    return (a @ w["down_w"][e].T) * wt[:, None]


# ---------------------------------------------------------------- kernel
def kernel(**inputs):
    w = {k: np.asarray(v, dtype=np.float32) for k, v in inputs.items()}
    x = w["x"][0]                                           # [S, H]

    if "nc0" not in _cache:
        _cache["nc0"] = build_neff0()
    in0 = _neff0_inputs(x, w)
    res0 = _run(_cache["nc0"], in0, "neff0")
    qn_raw = np.concatenate(
        [res0.results[c]["qn_out"] for c in range(NC)],
        axis=2).astype(np.float32)                        # [10, P, S]
    msq = np.concatenate([res0.results[c]["msq_out"] for c in range(NC)],
                         axis=2)                          # [2, 1, S]
    arow = ((x * x).sum(-1) / H + EPS).astype(np.float32)  # [S]
    sig_q = 1.0 / np.sqrt(msq[0, 0] / QR + EPS * arow)     # [S]
    sig_kv = 1.0 / np.sqrt(msq[1, 0] / KVR + EPS * arow)
    qn_raw[:6] *= sig_q[None, None, :]
    qn_raw[6:] *= sig_kv[None, None, :]
    qnT = qn_raw.astype(ml_dtypes.bfloat16)

    if "nc1" not in _cache:
        _cache["nc1"] = build_neff1()
    nc1 = _cache["nc1"]
    in1 = _neff1_inputs(w, qnT)
    res1 = _run(nc1, in1, "neff1")
    o_sum = np.zeros((S, H), np.float32)
    for c in range(NC):
        o_sum += res1.results[c]["o_part"].astype(np.float32)
    x2 = x + o_sum

    r2 = 1.0 / np.sqrt((x2 * x2).mean(-1, keepdims=True) + EPS)
    h2 = (x2 * r2).astype(np.float32)                       # rmsnorm w/o ln2
    h2ln = h2 * w["ln2_w"][None, :]
    order, wts = _route(h2ln, w)

    if "nc2" not in _cache:
        _cache["nc2"] = build_neff2()
    nc2 = _cache["nc2"]
    in2, idx_lists, wt_lists, spill = _neff2_inputs(h2ln, w, order, wts)
    res2 = _run(nc2, in2, "neff2")

    out = x2.copy()
    for c in range(NC):
        r = res2.results[c]
        for i in range(EPC):
            e = EPC * c + i
            tok = idx_lists[e][:CAP]
            ye = r["yrT"][i].reshape(H, CAP).T.astype(np.float32)  # [CAP, H]
            out[tok] += ye[:len(tok)]
        out[256 * c:256 * (c + 1)] += r["yshT"].reshape(H, 256).T.astype(
            np.float32)
    for e, tok, tw in spill:
        out[tok] += _expert_np(h2ln, tok, tw, w, e)
    return out.reshape(1, S, H).astype(np.float32)



# revision 23
# speedup vs baseline: 1.4488x; 1.0340x over previous
"""nn_DecoderLayer (MLA attention + MoE routing) on 8 TRN2 NeuronCores.

Strategy:
  NEFF1 (attention): head-parallel — core c computes heads {2c, 2c+1}:
    replicated q_a/kv_a down-projections (feature-major, fp32r matmuls),
    per-head q_b/kv_b + RoPE (rotate-half folded into host-augmented
    weights), causal scoresT [k,q] layout, exp softmax without max
    subtraction (|scores| ~ 1.5), AV accumulate, partial o-projection.
    Host sums the 8 o-partials (expert-parallel style combine), adds
    residual, computes rmsnorm + router + top-4 routing in numpy.
  NEFF2 (MoE): expert-parallel — core c owns experts {2c, 2c+1}: gathered
    per-expert token batches (capacity CAP) through gate/up/silu/down with
    the combine weight folded into the activation; shared expert is
    token-parallel (core c handles tokens [256c, 256c+256)).
  Host scatters expert outputs back, adds shared + residual.

All matmuls fp32r (measured ~1.5e-4 rel err on HW at full PE rate).
"""
import math
import ml_dtypes
import numpy as np

import concourse.bacc as bacc
import concourse.mybir as mybir
import concourse.tile as tile
from concourse import bass_utils
from concourse.bass import ts
from concourse.masks import make_identity

# problem dims
S, H = 2048, 2048
NH, NOPE, ROPE, DV = 16, 128, 64, 128
DQK = NOPE + ROPE                  # 192
QR, KVR = 768, 512
E, KTOP, MI = 16, 4, 1024
SCALE = 2.5
EPS = 1e-6
ROPE_BASE = 10000.0

NC = 8                              # cores
HPC = NH // NC                      # heads/core = 2
EPC = E // NC                       # experts/core = 2
CAP = 480                           # per-expert token capacity (overflow -> host)
P = 128
NSTRIP = S // 512                   # 4 strips of 512 tokens

F32 = mybir.dt.float32
F32R = mybir.dt.float32r
BF16 = mybir.dt.bfloat16

Exp = mybir.ActivationFunctionType.Exp
Sqrt = mybir.ActivationFunctionType.Sqrt
Square = mybir.ActivationFunctionType.Square
Silu = mybir.ActivationFunctionType.Silu
Identity = mybir.ActivationFunctionType.Identity

_cache = {}

# profiling hooks (test.py sets TRACE=True; harness leaves it False)
TRACE = False
PROF = {}


def _run(nc, in_maps, name):
    if TRACE:
        res = bass_utils.run_bass_kernel_spmd(
            nc, in_maps, core_ids=list(range(NC)), trace=True, trace_cores=[0])
        it = res.instructions_and_trace or (None, None)
        PROF[name] = {"exec_time_ns": res.exec_time_ns, "trace": it[1],
                      "insts": it[0]}
        return res
    return bass_utils.run_bass_kernel_spmd(nc, in_maps,
                                           core_ids=list(range(NC)),
                                           trace=False)


# ---------------------------------------------------------------- NEFF 0
def build_neff0():
    """Token-sharded q_a/kv_a down-projections: core c handles tokens
    [256c, 256c+256). Outputs raw feature-major qn/kvn (bf16) + per-token
    sum-of-squares rows (host finishes the rmsnorm sigma)."""
    nc = bacc.Bacc("TRN2", num_devices=NC, debug=False)
    def inp(name, shape):
        return nc.dram_tensor(name, list(shape), BF16, kind="ExternalInput").ap()

    xTs = inp("xTs", (P, 16, 256))                 # my tokens, feature-major
    qaw = inp("qaw", (P, 6, 16, P))                # lhsT: [p, m_tile, h_chunk, j]
    kvaw = inp("kvaw", (P, 4, 16, P))
    qn_out = nc.dram_tensor("qn_out", [10, P, 256], BF16,
                            kind="ExternalOutput").ap()
    msq_out = nc.dram_tensor("msq_out", [2, 1, 256], F32,
                             kind="ExternalOutput").ap()

    with tile.TileContext(nc) as tc:
        with tc.tile_pool(name="p0", bufs=1) as p0, \
             tc.tile_pool(name="p02", bufs=2) as p02, \
             tc.tile_pool(name="ps0", bufs=2, space="PSUM") as ps0, \
             tc.tile_pool(name="ps0r", bufs=1, space="PSUM") as ps0r:
            ones_f = p0.tile([P, 1], F32)
            nc.vector.memset(ones_f, 1.0)
            ones_r = p0.tile([P, 1], F32R)
            nc.scalar.copy(ones_r, ones_f)
            xs = p0.tile([P, 16, 256], BF16)
            nc.sync.dma_start(out=xs, in_=xTs)
            # per-m-tile weight tiles so the first matmul starts after 0.5MB
            wtiles = []
            for m in range(6):
                t = p0.tile([P, 16, P], BF16, tag=f"qaw{m}", name=f"qaw{m}")
                nc.sync.dma_start(out=t, in_=qaw[:, m])
                wtiles.append(t)
            for m in range(4):
                t = p0.tile([P, 16, P], BF16, tag=f"kvaw{m}", name=f"kvaw{m}")
                nc.sync.dma_start(out=t, in_=kvaw[:, m])
                wtiles.append(t)

            for kind in range(2):
                mt = 6 if kind == 0 else 4
                off = 0 if kind == 0 else 6
                msq_ps = ps0r.tile([1, 256], F32, tag="row")
                for m in range(mt):
                    mm_ps = ps0.tile([P, 256], F32, tag="mm")
                    for c in range(16):
                        nc.tensor.matmul(mm_ps, wtiles[off + m][:, c, :],
                                         xs[:, c, :],
                                         start=(c == 0), stop=(c == 15))
                    raw = p02.tile([P, 256], BF16, tag="raw")
                    nc.scalar.copy(raw, mm_ps)
                    nc.sync.dma_start(out=qn_out[off + m], in_=raw)
                    sq = p02.tile([P, 256], F32R, tag="sq")
                    nc.scalar.activation(out=sq, in_=mm_ps, func=Square)
                    nc.tensor.matmul(msq_ps, ones_r, sq,
                                     start=(m == 0), stop=(m == mt - 1))
                msq_row = p02.tile([1, 256], F32, tag="msqr")
                nc.scalar.copy(msq_row, msq_ps)
                nc.sync.dma_start(out=msq_out[kind], in_=msq_row)
    nc.compile()
    return nc


# ---------------------------------------------------------------- NEFF 1
def build_neff1():
    """Head-parallel attention, bf16 operands with fp32 PSUM.

    qnT arrives pre-scaled by the rmsnorm sigmas (host-folded), so window B
    is pure matmul + RoPE combine. Augmented weight layout per head:
    q: [nope 128 | rope 64; rot 64], kv: [kn 128 | kr 64; krot 64 | v 128]
    -> every matmul runs with M=128.
    """
    nc = bacc.Bacc("TRN2", num_devices=NC, debug=False)
    def inp(name, shape, dt=BF16):
        return nc.dram_tensor(name, list(shape), dt, kind="ExternalInput").ap()

    qnT_i = inp("qnT", (10, P, S))                 # feature-major qn/kvn (scaled)
    qbw = inp("qbw", (P, 6, 512))                  # lhsT: [p, qr_chunk, m]
    kvbw = inp("kvbw", (P, 4, 768))                # lhsT: [p, kvr_chunk, m]
    ow = inp("ow", (P, HPC, H))                    # rhs: [p, c_chunk, h]
    cosT = inp("cosT", (ROPE, S), F32)
    sinT = inp("sinT", (ROPE, S), F32)
    masks = inp("masks", (P, 4, 512))              # tri01 keep-mask per diag offset
    o_part = nc.dram_tensor("o_part", [S, H], BF16, kind="ExternalOutput").ap()

    with tile.TileContext(nc) as tc:
        with tc.tile_pool(name="const", bufs=1) as cpool, \
             tc.tile_pool(name="dram", bufs=1, space="DRAM") as dr:

            ones_f = cpool.tile([P, 1], F32)
            nc.vector.memset(ones_f, 1.0)
            ones_b = cpool.tile([P, 1], BF16)
            nc.scalar.copy(ones_b, ones_f)
            onesrow_f = cpool.tile([1, P], F32)
            nc.vector.memset(onesrow_f, 1.0)
            onesrow_r = cpool.tile([1, P], F32R)
            nc.scalar.copy(onesrow_r, onesrow_f)
            ident = cpool.tile([P, P], BF16)
            make_identity(nc, ident)

            # ---------------- Windows B + C under the resident pool
            with tc.tile_pool(name="res", bufs=1) as res:
              qf_n = res.tile([P, HPC, S], BF16)
              qf_r = res.tile([64, HPC, S], BF16)
              kf_n = res.tile([P, HPC, S], BF16)
              kf_r = res.tile([64, HPC, S], BF16)
              v_sb = res.tile([P, HPC, 16, DV], BF16)
              # ---------------- Window B: q_b / kv_b + RoPE -> resident qf/kf/v
              with tc.tile_pool(name="pb", bufs=1) as pb, \
                   tc.tile_pool(name="pb2", bufs=2) as pb2, \
                   tc.tile_pool(name="pb1", bufs=2) as pb1, \
                   tc.tile_pool(name="psB", bufs=2, space="PSUM") as psB:
                  qbw_sb = pb.tile([P, 6, 512], BF16)
                  nc.sync.dma_start(out=qbw_sb, in_=qbw)
                  kvbw_sb = pb.tile([P, 4, 768], BF16)
                  nc.sync.dma_start(out=kvbw_sb, in_=kvbw)
                  cos_sb = pb.tile([ROPE, S], F32)
                  nc.sync.dma_start(out=cos_sb, in_=cosT)
                  sin_sb = pb.tile([ROPE, S], F32)
                  nc.sync.dma_start(out=sin_sb, in_=sinT)

                  for s in range(NSTRIP):
                      sl = slice(512 * s, 512 * (s + 1))
                      # 5 chunk-pair tiles so matmuls start after 256KB
                      chunks = []
                      for g in range(5):
                          qn_t = pb2.tile([P, 2, 512], BF16, tag=f"qnt{g}",
                                          name=f"qnt{g}")
                          nc.sync.dma_start(
                              out=qn_t,
                              in_=qnT_i[2 * g:2 * g + 2, :, sl].rearrange(
                                  "c p j -> p c j"))
                          chunks.extend([qn_t[:, 0, :], qn_t[:, 1, :]])
                      qn = chunks[:6]
                      kvn = chunks[6:]
                      for hi in range(HPC):
                          for side in range(2):  # 0: q, 1: k/v
                              if side == 0:
                                  wsb, chunks, base, nmt = qbw_sb, qn, 256 * hi, 6
                                  dn, rr = qf_n, qf_r
                              else:
                                  wsb, chunks, base, nmt = kvbw_sb, kvn, 384 * hi, 4
                                  dn, rr = kf_n, kf_r
                              ps_n = psB.tile([P, 512], F32, tag="bn")
                              ps_ror = psB.tile([P, 512], F32, tag="bro")
                              for c in range(nmt):
                                  st, sp = (c == 0), (c == nmt - 1)
                                  nc.tensor.matmul(ps_n, wsb[:, c, base:base + 128],
                                                   chunks[c], start=st, stop=sp)
                                  nc.tensor.matmul(ps_ror,
                                                   wsb[:, c, base + 128:base + 256],
                                                   chunks[c], start=st, stop=sp)
                              nc.scalar.copy(dn[:, hi, sl], ps_n)
                              t1 = pb1.tile([64, 512], F32, tag="t1")
                              nc.vector.tensor_mul(t1, ps_ror[0:64, :],
                                                   cos_sb[:, sl])
                              t2 = pb1.tile([64, 512], F32, tag="t2")
                              nc.vector.tensor_mul(t2, ps_ror[64:128, :],
                                                   sin_sb[:, sl])
                              nc.vector.tensor_add(rr[:, hi, sl], t1, t2)
                              if side == 1:
                                  # v feature-major, then PE transpose
                                  ps_v = psB.tile([P, 512], F32, tag="bv")
                                  for c in range(4):
                                      nc.tensor.matmul(
                                          ps_v, kvbw_sb[:, c, base + 256:base + 384],
                                          chunks[c], start=(c == 0), stop=(c == 3))
                                  v_fm = pb1.tile([P, 512], BF16, tag="vfm")
                                  nc.scalar.copy(v_fm, ps_v)
                                  for t in range(4):
                                      ps_t = psB.tile([P, P], BF16, tag="bt")
                                      nc.tensor.transpose(ps_t, v_fm[:, ts(t, P)],
                                                          ident)
                                      nc.scalar.copy(v_sb[:, hi, 4 * s + t, :], ps_t)

              # ---------------- Window C: attention + o-projection
              with tc.tile_pool(name="pc", bufs=1) as pc, \
                   tc.tile_pool(name="pc2", bufs=2) as pc2, \
                   tc.tile_pool(name="pc3", bufs=3) as pc3, \
                   tc.tile_pool(name="psC", bufs=2, space="PSUM") as psC, \
                   tc.tile_pool(name="psD", bufs=2, space="PSUM") as psD:
                  mask_sb = pc.tile([P, 4, 512], BF16)
                  nc.sync.dma_start(out=mask_sb, in_=masks)
                  ow_sb = pc.tile([P, HPC, H], BF16)
                  nc.sync.dma_start(out=ow_sb, in_=ow)

                  for s in range(NSTRIP):
                      sl = slice(512 * s, 512 * (s + 1))
                      ctx_sb = pc2.tile([P, HPC, 512], BF16, tag="ctx")
                      for hi in range(HPC):
                          ps_ctx = psD.tile([P, 512], F32, tag="ctx")
                          ps_den = psD.tile([1, 512], F32, tag="den")
                          nkc = 4 * s + 4
                          for kc in range(nkc):
                              ps_sc = psC.tile([P, 512], F32, tag="sc")
                              nc.tensor.matmul(ps_sc, kf_n[:, hi, ts(kc, P)],
                                               qf_n[:, hi, sl],
                                               start=True, stop=False)
                              nc.tensor.matmul(ps_sc, kf_r[:, hi, ts(kc, P)],
                                               qf_r[:, hi, sl],
                                               start=False, stop=True)
                              att = pc3.tile([P, 512], BF16, tag="att")
                              nc.scalar.activation(out=att, in_=ps_sc, func=Exp)
                              if kc >= 4 * s:
                                  nc.gpsimd.tensor_mul(att, att,
                                                       mask_sb[:, kc - 4 * s, :])
                              nc.tensor.matmul(ps_den, ones_b, att,
                                               start=(kc == 0), stop=(kc == nkc - 1))
                              nc.tensor.matmul(ps_ctx, v_sb[:, hi, kc, :], att,
                                               start=(kc == 0), stop=(kc == nkc - 1))
                          rcp = pc2.tile([1, 512], F32, tag="rcp")
                          nc.vector.reciprocal_approx_fast(out=rcp, in_=ps_den)
                          rcp_r = pc2.tile([1, 512], F32R, tag="rcpr")
                          nc.scalar.copy(rcp_r, rcp)
                          # broadcast 1/den across partitions via PE rank-1
                          ps_rb = psC.tile([P, 512], F32, tag="o")
                          nc.tensor.matmul(ps_rb, onesrow_r, rcp_r,
                                           start=True, stop=True)
                          rcp_bc = pc2.tile([P, 512], F32, tag="rbc")
                          nc.scalar.copy(rcp_bc, ps_rb)
                          nc.vector.tensor_mul(ctx_sb[:, hi, :], ps_ctx, rcp_bc)
                      # o-projection for this strip (token-major out)
                      for t in range(4):
                          o_sb = pc2.tile([P, H], BF16, tag="osb")
                          for hs in range(4):
                              ps_o = psC.tile([P, 512], F32, tag="o")
                              for cc in range(HPC):
                                  nc.tensor.matmul(ps_o, ctx_sb[:, cc, ts(t, P)],
                                                   ow_sb[:, cc, ts(hs, 512)],
                                                   start=(cc == 0), stop=(cc == HPC - 1))
                              nc.vector.tensor_copy(o_sb[:, ts(hs, 512)], ps_o)
                          nc.sync.dma_start(
                              out=o_part.rearrange("(T p) h -> p T h", p=P)[:, 4 * s + t, :],
                              in_=o_sb)
    nc.compile()
    return nc


# ---------------------------------------------------------------- NEFF 2
def build_neff2():
    """Expert-parallel MoE, all-bf16 weights/activations (fp32 PSUM accum).

    Per expert slot (2 + shared): 8 gate/up m-tiles of [P,2048]->CAP
    through silu*up*combine, then 16 down h-tiles streamed [P,8,P].
    CAP=512 -> single PSUM strip per tile.
    """
    nc = bacc.Bacc("TRN2", num_devices=NC, debug=False)
    def inp(name, shape):
        return nc.dram_tensor(name, list(shape), BF16, kind="ExternalInput").ap()

    xe = inp("xe", (EPC, 16, P, CAP))        # gathered expert tokens (h2ln)
    gw = inp("gw", (EPC, 8, P, 2048))        # gate lhsT prepack
    uw = inp("uw", (EPC, 8, P, 2048))
    dw = inp("dw", (EPC, 16, P, 8, P))       # down lhsT per h-tile
    wrow = inp("wrow", (EPC, 1, CAP))        # combine weights (row layout)
    h2t = inp("h2t", (16, P, 256))           # my 256 tokens, feature-major
    sgw = inp("sgw", (8, P, 2048))
    suw = inp("suw", (8, P, 2048))
    sdw = inp("sdw", (16, P, 8, P))
    yrT = nc.dram_tensor("yrT", [EPC, 16, P, CAP], BF16,
                         kind="ExternalOutput").ap()
    yshT = nc.dram_tensor("yshT", [16, P, 256], BF16,
                          kind="ExternalOutput").ap()

    with tile.TileContext(nc) as tc:
        with tc.tile_pool(name="px", bufs=2) as px, \
             tc.tile_pool(name="pw", bufs=3) as pw, \
             tc.tile_pool(name="pd", bufs=3) as pd, \
             tc.tile_pool(name="pact", bufs=2) as pact, \
             tc.tile_pool(name="py", bufs=3) as py, \
             tc.tile_pool(name="ps", bufs=2, space="PSUM") as ps, \
             tc.tile_pool(name="psy", bufs=2, space="PSUM") as psy:

            def expert_block(xe_sb, w_bc, gw_i, uw_i, dw_i, y_out, n):
                """One expert's gate/up/silu/down over n token slots."""
                act = pact.tile([P, 8, CAP], BF16, tag="act")
                for t in range(8):
                    g_w = pw.tile([P, 16, P], BF16, tag="gw")
                    nc.sync.dma_start(out=g_w, in_=gw_i[t].rearrange(
                        "p (hc j) -> p hc j", j=P))
                    u_w = pw.tile([P, 16, P], BF16, tag="uw")
                    nc.sync.dma_start(out=u_w, in_=uw_i[t].rearrange(
                        "p (hc j) -> p hc j", j=P))
                    ps_g = ps.tile([P, n], F32, tag="g")
                    ps_u = ps.tile([P, n], F32, tag="u")
                    for hc in range(16):
                        st, sp = (hc == 0), (hc == 15)
                        nc.tensor.matmul(ps_g, g_w[:, hc, :], xe_sb[:, hc, 0:n],
                                         start=st, stop=sp)
                        nc.tensor.matmul(ps_u, u_w[:, hc, :], xe_sb[:, hc, 0:n],
                                         start=st, stop=sp)
                    sil = pw.tile([P, n], F32, tag="sil")
                    nc.scalar.activation(out=sil, in_=ps_g, func=Silu)
                    if w_bc is not None:
                        nc.gpsimd.tensor_mul(sil, sil, w_bc[:, 0:n])
                    nc.vector.tensor_mul(act[:, t, 0:n], sil, ps_u)
                for ht in range(16):
                    d_w = pd.tile([P, 8, P], BF16, tag="dw")
                    nc.sync.dma_start(out=d_w, in_=dw_i[ht])
                    ps_y = psy.tile([P, n], F32, tag="y")
                    for mc in range(8):
                        nc.tensor.matmul(ps_y, d_w[:, mc, :], act[:, mc, 0:n],
                                         start=(mc == 0), stop=(mc == 7))
                    y_sb = py.tile([P, n], BF16, tag="ysb")
                    nc.scalar.copy(y_sb, ps_y)
                    nc.sync.dma_start(out=y_out[ht], in_=y_sb)

            for i in range(EPC):
                xe_sb = px.tile([P, 16, CAP], BF16, tag="xe")
                for qc in range(4):
                    nc.sync.dma_start(
                        out=xe_sb[:, 4 * qc:4 * qc + 4, :],
                        in_=xe[i, 4 * qc:4 * qc + 4].rearrange(
                            "hc p t -> p hc t"))
                w_bc = px.tile([P, CAP], BF16, tag="wbc")
                nc.sync.dma_start(out=w_bc, in_=wrow[i].broadcast_to((P, CAP)))
                expert_block(xe_sb, w_bc, gw[i], uw[i], dw[i], yrT[i], CAP)

            # ---------------- shared expert (my 256 tokens)
            h2_sb = px.tile([P, 16, 256], BF16, tag="h2")
            nc.sync.dma_start(out=h2_sb, in_=h2t.rearrange("hc p t -> p hc t"))
            expert_block(h2_sb, None, sgw, suw, sdw, yshT, 256)
    nc.compile()
    return nc


# ---------------------------------------------------------------- host prep
def _rope_tables():
    inv = 1.0 / (ROPE_BASE ** (np.arange(0, ROPE, 2, dtype=np.float64) / ROPE))
    t = np.arange(S, dtype=np.float64)
    f = t[:, None] * inv[None, :]
    emb = np.concatenate([f, f], axis=-1)          # [S, 64]
    return (np.cos(emb).T.astype(np.float32).copy(),
            np.sin(emb).T.astype(np.float32).copy())


def _lhsT_prepack(wT, mtiles):
    """wT [K, M] -> [P, mtiles, K//P, P]: SBUF-image for resident lhsT tiles."""
    Kd, Md = wT.shape
    assert Md == mtiles * P and Kd % P == 0
    return np.ascontiguousarray(
        wT.reshape(Kd // P, P, mtiles, P).transpose(1, 2, 0, 3))


def _lhsT_prepack2(wT, mtiles):
    """wT [K, M] -> [mtiles, P, K]: per-m-tile contiguous DMA layout.

    Tile t, flattened [P, K] with per-partition layout (hc, j):
    A[t, p, 128*hc + j] = wT[128*hc + p, 128*t + j].
    """
    Kd, Md = wT.shape
    assert Md == mtiles * P and Kd % P == 0
    return np.ascontiguousarray(
        wT.reshape(Kd // P, P, mtiles, P).transpose(2, 1, 0, 3).reshape(
            mtiles, P, Kd))


def _neff0_inputs(x, w):
    bf = ml_dtypes.bfloat16
    ln1 = w["ln1_w"]
    xT = x.T.astype(np.float32)                                   # [H, S]
    WqT = (w["q_a_w"] * ln1[None, :]).T.astype(np.float32)        # [H, QR]
    WkvT = (w["kv_a_w"] * ln1[None, :]).T.astype(np.float32)      # [H, KVR]
    qaw = _lhsT_prepack(WqT, 6).astype(bf)
    kvaw = _lhsT_prepack(WkvT, 4).astype(bf)
    per_core = []
    for c in range(NC):
        rows = slice(256 * c, 256 * (c + 1))
        xTs = np.ascontiguousarray(xT[:, rows].reshape(16, P, 256)
                                   .transpose(1, 0, 2)).astype(bf)
        per_core.append({"xTs": xTs, "qaw": qaw, "kvaw": kvaw})
    return per_core


def _neff1_inputs(w, qnT):
    bf = ml_dtypes.bfloat16
    qb = (w["q_b_w"] * w["q_a_ln"][None, :]).astype(np.float32)   # [NH*DQK, QR]
    kvb = (w["kv_b_w"] * w["kv_a_ln"][None, :]).astype(np.float32)  # [NH*320, KVR]
    sc = 1.0 / math.sqrt(DQK)
    cosT, sinT = _rope_tables()

    masks = np.zeros((P, 4, 512), np.float32)
    pp, jj = np.meshgrid(np.arange(P), np.arange(512), indexing="ij")
    for cl in range(4):
        masks[:, cl, :] = (P * cl + pp <= jj).astype(np.float32)
    masks = masks.astype(bf)

    per_core = []
    for c in range(NC):
        heads = [HPC * c + i for i in range(HPC)]
        # q_b augmented: per head rows [nope 128 | rope 64 | rot 64], scaled by sc
        qrows = []
        for h in heads:
            blk = qb[h * DQK:(h + 1) * DQK] * sc                   # [192, QR]
            nope, rope = blk[:NOPE], blk[NOPE:]
            rot = np.concatenate([-rope[32:], rope[:32]], axis=0)
            qrows.append(np.concatenate([nope, rope, rot], axis=0))  # [256, QR]
        qaug = np.concatenate(qrows, axis=0)                       # [512, QR]
        qbw = np.ascontiguousarray(qaug.T.reshape(6, P, 512).transpose(1, 0, 2))

        kvrows = []
        for h in heads:
            blk = kvb[h * 320:(h + 1) * 320]                       # [320, KVR]
            kn, kr, vv = blk[:NOPE], blk[NOPE:DQK], blk[DQK:]
            krot = np.concatenate([-kr[32:], kr[:32]], axis=0)
            kvrows.append(np.concatenate([kn, kr, krot, vv], axis=0))  # [384, KVR]
        kvaug = np.concatenate(kvrows, axis=0)                     # [768, KVR]
        kvbw = np.ascontiguousarray(kvaug.T.reshape(4, P, 768).transpose(1, 0, 2))

        ocols = np.concatenate([w["o_w"][:, h * DV:(h + 1) * DV] for h in heads],
                               axis=1)                             # [H, 256]
        owp = np.ascontiguousarray(ocols.T.reshape(HPC, P, H).transpose(1, 0, 2))

        per_core.append({
            "qnT": qnT,
            "qbw": qbw.astype(bf), "kvbw": kvbw.astype(bf),
            "ow": owp.astype(bf), "cosT": cosT, "sinT": sinT,
            "masks": masks,
        })
    return per_core


def _rmsn(v, eps=EPS):
    return v / np.sqrt((v * v).mean(-1, keepdims=True) + eps)


def _exact_probs(x, w, tb):
    """fp32 numpy router probs for tokens tb: exact attention rows ->
    exact x2 rows -> exact logits. Borderline-token routing must match the
    fp32 reference; the device's bf16 x2 is too coarse for near-ties."""
    h1 = _rmsn(x) * w["ln1_w"]
    kvl = _rmsn(h1 @ w["kv_a_w"].T) * w["kv_a_ln"]
    kvp = (kvl @ w["kv_b_w"].T).reshape(S, NH, DQK + DV)
    kn, kr, vv = kvp[..., :NOPE], kvp[..., NOPE:DQK], kvp[..., DQK:]
    inv = 1.0 / (ROPE_BASE ** (np.arange(0, ROPE, 2, dtype=np.float64) / ROPE))
    f = np.arange(S, dtype=np.float64)[:, None] * inv[None, :]
    emb = np.concatenate([f, f], axis=-1)
    cos = np.cos(emb).astype(np.float32)[:, None, :]
    sin = np.sin(emb).astype(np.float32)[:, None, :]
    rot = lambda z: np.concatenate([-z[..., 32:], z[..., :32]], axis=-1)
    kf = np.concatenate([kn, kr * cos + rot(kr) * sin], axis=-1)  # [S,NH,192]

    h1b = h1[tb]
    ql = _rmsn(h1b @ w["q_a_w"].T) * w["q_a_ln"]
    qp = (ql @ w["q_b_w"].T).reshape(len(tb), NH, DQK)
    qn_, qr_ = qp[..., :NOPE], qp[..., NOPE:]
    qr_ = qr_ * cos[tb] + rot(qr_) * sin[tb]
    qf = np.concatenate([qn_, qr_], axis=-1)                      # [B,NH,192]

    sc = np.einsum("bhd,khd->bhk", qf, kf) / math.sqrt(DQK)
    keymask = np.arange(S)[None, :] > tb[:, None]                 # [B,S]
    sc = sc - 1e9 * keymask[:, None, :]
    sc -= sc.max(-1, keepdims=True)
    att = np.exp(sc)
    att /= att.sum(-1, keepdims=True)
    ctx = np.einsum("bhk,khd->bhd", att, vv).reshape(len(tb), NH * DV)
    x2r = x[tb] + ctx @ w["o_w"].T
    h2r = _rmsn(x2r) * w["ln2_w"]
    logits = h2r @ w["router_w"].T + w["router_b"][None, :]
    return 1.0 / (1.0 + np.exp(-logits))


def _route(h2ln, w, x):
    """Top-4 routing in numpy; borderline tokens (4th vs 5th prob near-tie)
    get their probs recomputed exactly so the discrete choice matches the
    fp32 reference."""
    logits = h2ln @ w["router_w"].T.astype(np.float32) + w["router_b"][None, :]
    probs = 1.0 / (1.0 + np.exp(-logits))
    sp = np.sort(probs, axis=-1)
    gap = sp[:, -KTOP] - sp[:, -KTOP - 1]
    tb = np.where(gap < 0.01)[0]
    if len(tb):
        probs[tb] = _exact_probs(x, w, tb)
    order = np.argsort(-probs, axis=-1, kind="stable")[:, :KTOP]
    topv = np.take_along_axis(probs, order, axis=-1)
    wts = topv / (topv.sum(-1, keepdims=True) + 1e-9) * SCALE
    return order, wts


def _down_prepack(dwT):
    """dwT [M, H] -> [16, P, 8, P]: per-h-tile lhsT chunks
    out[ht, p, mc, j] = dwT[mc*128+p, ht*128+j]."""
    return np.ascontiguousarray(
        dwT.reshape(8, P, 16, P).transpose(2, 1, 0, 3))


def _neff2_inputs(h2ln, w, order, wts):
    """Expert batches gathered from h2ln (ln2 pre-applied); bf16 payloads."""
    bf = ml_dtypes.bfloat16
    idx_lists, wt_lists = [], []
    for e in range(E):
        tok, kk = np.where(order == e)
        idx_lists.append(tok)
        wt_lists.append(wts[tok, kk])

    h2T = np.ascontiguousarray(h2ln.T).astype(bf)           # [H, S]
    per_core = []
    spill = []                                              # (expert, tok, wt) overflow
    sgw = _lhsT_prepack2(w["sg_w"].T.astype(np.float32), 8).astype(bf)
    suw = _lhsT_prepack2(w["su_w"].T.astype(np.float32), 8).astype(bf)
    sdw = _down_prepack(w["sd_w"].T).astype(bf)

    for c in range(NC):
        xeb = np.zeros((EPC, 16, P, CAP), bf)
        gwb = np.zeros((EPC, 8, P, 2048), bf)
        uwb = np.zeros((EPC, 8, P, 2048), bf)
        dwb = np.zeros((EPC, 16, P, 8, P), bf)
        wrow = np.zeros((EPC, 1, CAP), bf)
        for i in range(EPC):
            e = EPC * c + i
            tok, tw = idx_lists[e], wt_lists[e]
            if len(tok) > CAP:
                spill.append((e, tok[CAP:], tw[CAP:]))
                tok, tw = tok[:CAP], tw[:CAP]
            n = len(tok)
            xeb[i, :, :, :n] = h2T[:, tok].reshape(16, P, n)
            wrow[i, 0, :n] = tw.astype(bf)
            gwb[i] = _lhsT_prepack2(w["gate_w"][e].T.astype(np.float32), 8)
            uwb[i] = _lhsT_prepack2(w["up_w"][e].T.astype(np.float32), 8)
            dwb[i] = _down_prepack(w["down_w"][e].T)
        rows = slice(256 * c, 256 * (c + 1))
        h2tp = np.ascontiguousarray(h2T[:, rows].reshape(16, P, 256))
        per_core.append({
            "xe": xeb, "gw": gwb, "uw": uwb, "dw": dwb, "wrow": wrow,
            "h2t": h2tp, "sgw": sgw, "suw": suw, "sdw": sdw,
        })
    return per_core, idx_lists, wt_lists, spill


def _expert_np(h2ln, idx, wt, w, e):
    """Numpy fallback for capacity-overflow tokens."""
    xg = h2ln[idx]
    g = xg @ w["gate_w"][e].T
    u = xg @ w["up_w"][e].T
    a = (g / (1 + np.exp(-g))) * u
    return (a @ w["down_w"][e].T) * wt[:, None]


# ---------------------------------------------------------------- kernel
def kernel(**inputs):
    w = {k: np.asarray(v, dtype=np.float32) for k, v in inputs.items()}
    x = w["x"][0]                                           # [S, H]

    if "nc0" not in _cache:
        _cache["nc0"] = build_neff0()
    in0 = _neff0_inputs(x, w)
    res0 = _run(_cache["nc0"], in0, "neff0")
    qn_raw = np.concatenate(
        [res0.results[c]["qn_out"] for c in range(NC)],
        axis=2).astype(np.float32)                        # [10, P, S]
    msq = np.concatenate([res0.results[c]["msq_out"] for c in range(NC)],
                         axis=2)                          # [2, 1, S]
    arow = ((x * x).sum(-1) / H + EPS).astype(np.float32)  # [S]
    sig_q = 1.0 / np.sqrt(msq[0, 0] / QR + EPS * arow)     # [S]
    sig_kv = 1.0 / np.sqrt(msq[1, 0] / KVR + EPS * arow)
    qn_raw[:6] *= sig_q[None, None, :]
    qn_raw[6:] *= sig_kv[None, None, :]
    qnT = qn_raw.astype(ml_dtypes.bfloat16)

    if "nc1" not in _cache:
        _cache["nc1"] = build_neff1()
    nc1 = _cache["nc1"]
    in1 = _neff1_inputs(w, qnT)
    res1 = _run(nc1, in1, "neff1")
    o_sum = np.zeros((S, H), np.float32)
    for c in range(NC):
        o_sum += res1.results[c]["o_part"].astype(np.float32)
    x2 = x + o_sum

    r2 = 1.0 / np.sqrt((x2 * x2).mean(-1, keepdims=True) + EPS)
    h2 = (x2 * r2).astype(np.float32)                       # rmsnorm w/o ln2
    h2ln = h2 * w["ln2_w"][None, :]
    order, wts = _route(h2ln, w)

    if "nc2" not in _cache:
        _cache["nc2"] = build_neff2()
    nc2 = _cache["nc2"]
    in2, idx_lists, wt_lists, spill = _neff2_inputs(h2ln, w, order, wts)
    res2 = _run(nc2, in2, "neff2")

    out = x2.copy()
    for c in range(NC):
        r = res2.results[c]
        for i in range(EPC):
            e = EPC * c + i
            tok = idx_lists[e][:CAP]
            ye = r["yrT"][i].reshape(H, CAP).T.astype(np.float32)  # [CAP, H]
            out[tok] += ye[:len(tok)]
        out[256 * c:256 * (c + 1)] += r["yshT"].reshape(H, 256).T.astype(
            np.float32)
    for e, tok, tw in spill:
        out[tok] += _expert_np(h2ln, tok, tw, w, e)
    return out.reshape(1, S, H).astype(np.float32)



# revision 29
# speedup vs baseline: 1.4765x; 1.0191x over previous
"""nn_DecoderLayer (MLA attention + MoE routing) on 8 TRN2 NeuronCores.

Strategy:
  NEFF1 (attention): head-parallel — core c computes heads {2c, 2c+1}:
    replicated q_a/kv_a down-projections (feature-major, fp32r matmuls),
    per-head q_b/kv_b + RoPE (rotate-half folded into host-augmented
    weights), causal scoresT [k,q] layout, exp softmax without max
    subtraction (|scores| ~ 1.5), AV accumulate, partial o-projection.
    Host sums the 8 o-partials (expert-parallel style combine), adds
    residual, computes rmsnorm + router + top-4 routing in numpy.
  NEFF2 (MoE): expert-parallel — core c owns experts {2c, 2c+1}: gathered
    per-expert token batches (capacity CAP) through gate/up/silu/down with
    the combine weight folded into the activation; shared expert is
    token-parallel (core c handles tokens [256c, 256c+256)).
  Host scatters expert outputs back, adds shared + residual.

All matmuls fp32r (measured ~1.5e-4 rel err on HW at full PE rate).
"""
import math
import ml_dtypes
import numpy as np

import concourse.bacc as bacc
import concourse.mybir as mybir
import concourse.tile as tile
from concourse import bass_utils
from concourse.bass import ts
from concourse.masks import make_identity

# problem dims
S, H = 2048, 2048
NH, NOPE, ROPE, DV = 16, 128, 64, 128
DQK = NOPE + ROPE                  # 192
QR, KVR = 768, 512
E, KTOP, MI = 16, 4, 1024
SCALE = 2.5
EPS = 1e-6
ROPE_BASE = 10000.0

NC = 8                              # cores
HPC = NH // NC                      # heads/core = 2
EPC = E // NC                       # experts/core = 2
CAP = 480                           # per-expert token capacity (overflow -> host)
P = 128
NSTRIP = S // 512                   # 4 strips of 512 tokens

F32 = mybir.dt.float32
F32R = mybir.dt.float32r
BF16 = mybir.dt.bfloat16

Exp = mybir.ActivationFunctionType.Exp
Sqrt = mybir.ActivationFunctionType.Sqrt
Square = mybir.ActivationFunctionType.Square
Silu = mybir.ActivationFunctionType.Silu
Identity = mybir.ActivationFunctionType.Identity

_cache = {}

# profiling hooks (test.py sets TRACE=True; harness leaves it False)
TRACE = False
PROF = {}


def _run(nc, in_maps, name):
    if TRACE:
        res = bass_utils.run_bass_kernel_spmd(
            nc, in_maps, core_ids=list(range(NC)), trace=True, trace_cores=[0])
        it = res.instructions_and_trace or (None, None)
        PROF[name] = {"exec_time_ns": res.exec_time_ns, "trace": it[1],
                      "insts": it[0]}
        return res
    return bass_utils.run_bass_kernel_spmd(nc, in_maps,
                                           core_ids=list(range(NC)),
                                           trace=False)


# ---------------------------------------------------------------- NEFF 0
def build_neff0():
    """Token-sharded q_a/kv_a down-projections: core c handles tokens
    [256c, 256c+256). Outputs raw feature-major qn/kvn (bf16) + per-token
    sum-of-squares rows (host finishes the rmsnorm sigma)."""
    nc = bacc.Bacc("TRN2", num_devices=NC, debug=False)
    def inp(name, shape):
        return nc.dram_tensor(name, list(shape), BF16, kind="ExternalInput").ap()

    xTs = inp("xTs", (P, 16, 256))                 # my tokens, feature-major
    qaw = inp("qaw", (P, 6, 16, P))                # lhsT: [p, m_tile, h_chunk, j]
    kvaw = inp("kvaw", (P, 4, 16, P))
    qn_out = nc.dram_tensor("qn_out", [10, P, 256], BF16,
                            kind="ExternalOutput").ap()
    msq_out = nc.dram_tensor("msq_out", [2, 1, 256], F32,
                             kind="ExternalOutput").ap()

    with tile.TileContext(nc) as tc:
        with tc.tile_pool(name="p0", bufs=1) as p0, \
             tc.tile_pool(name="p02", bufs=2) as p02, \
             tc.tile_pool(name="ps0", bufs=2, space="PSUM") as ps0, \
             tc.tile_pool(name="ps0r", bufs=1, space="PSUM") as ps0r:
            ones_f = p0.tile([P, 1], F32)
            nc.vector.memset(ones_f, 1.0)
            ones_r = p0.tile([P, 1], F32R)
            nc.scalar.copy(ones_r, ones_f)
            xs = p0.tile([P, 16, 256], BF16)
            nc.sync.dma_start(out=xs, in_=xTs)
            # per-m-tile weight tiles so the first matmul starts after 0.5MB;
            # spread DGE issue across idle engine sequencers
            issuers = [nc.scalar, nc.sync, nc.gpsimd]
            wtiles = []
            for m in range(6):
                t = p0.tile([P, 16, P], BF16, tag=f"qaw{m}", name=f"qaw{m}")
                issuers[m % 3].dma_start(out=t, in_=qaw[:, m])
                wtiles.append(t)
            for m in range(4):
                t = p0.tile([P, 16, P], BF16, tag=f"kvaw{m}", name=f"kvaw{m}")
                issuers[m % 3].dma_start(out=t, in_=kvaw[:, m])
                wtiles.append(t)

            for kind in range(2):
                mt = 6 if kind == 0 else 4
                off = 0 if kind == 0 else 6
                msq_ps = ps0r.tile([1, 256], F32, tag="row")
                for m in range(mt):
                    mm_ps = ps0.tile([P, 256], F32, tag="mm")
                    for c in range(16):
                        nc.tensor.matmul(mm_ps, wtiles[off + m][:, c, :],
                                         xs[:, c, :],
                                         start=(c == 0), stop=(c == 15))
                    raw = p02.tile([P, 256], BF16, tag="raw")
                    nc.scalar.copy(raw, mm_ps)
                    nc.sync.dma_start(out=qn_out[off + m], in_=raw)
                    sq = p02.tile([P, 256], F32R, tag="sq")
                    nc.scalar.activation(out=sq, in_=mm_ps, func=Square)
                    nc.tensor.matmul(msq_ps, ones_r, sq,
                                     start=(m == 0), stop=(m == mt - 1))
                msq_row = p02.tile([1, 256], F32, tag="msqr")
                nc.scalar.copy(msq_row, msq_ps)
                nc.sync.dma_start(out=msq_out[kind], in_=msq_row)
    nc.compile()
    return nc


# ---------------------------------------------------------------- NEFF 1
def build_neff1():
    """Head-parallel attention, bf16 operands with fp32 PSUM.

    qnT arrives pre-scaled by the rmsnorm sigmas (host-folded), so window B
    is pure matmul + RoPE combine. Augmented weight layout per head:
    q: [nope 128 | rope 64; rot 64], kv: [kn 128 | kr 64; krot 64 | v 128]
    -> every matmul runs with M=128.
    """
    nc = bacc.Bacc("TRN2", num_devices=NC, debug=False)
    def inp(name, shape, dt=BF16):
        return nc.dram_tensor(name, list(shape), dt, kind="ExternalInput").ap()

    qnT_i = inp("qnT", (10, P, S))                 # feature-major qn/kvn (scaled)
    qbw = inp("qbw", (P, 6, 512))                  # lhsT: [p, qr_chunk, m]
    kvbw = inp("kvbw", (P, 4, 768))                # lhsT: [p, kvr_chunk, m]
    ow = inp("ow", (P, HPC, H))                    # rhs: [p, c_chunk, h]
    cosT = inp("cosT", (ROPE, S), F32)
    sinT = inp("sinT", (ROPE, S), F32)
    masks = inp("masks", (P, 4, 512))              # tri01 keep-mask per diag offset
    o_part = nc.dram_tensor("o_part", [S, H], BF16, kind="ExternalOutput").ap()

    with tile.TileContext(nc) as tc:
        with tc.tile_pool(name="const", bufs=1) as cpool, \
             tc.tile_pool(name="dram", bufs=1, space="DRAM") as dr:

            ones_f = cpool.tile([P, 1], F32)
            nc.vector.memset(ones_f, 1.0)
            ones_b = cpool.tile([P, 1], BF16)
            nc.scalar.copy(ones_b, ones_f)
            onesrow_f = cpool.tile([1, P], F32)
            nc.vector.memset(onesrow_f, 1.0)
            onesrow_r = cpool.tile([1, P], F32R)
            nc.scalar.copy(onesrow_r, onesrow_f)
            ident = cpool.tile([P, P], BF16)
            make_identity(nc, ident)

            # ---------------- Windows B + C under the resident pool
            with tc.tile_pool(name="res", bufs=1) as res:
              qf_n = res.tile([P, HPC, S], BF16)
              qf_r = res.tile([64, HPC, S], BF16)
              kf_n = res.tile([P, HPC, S], BF16)
              kf_r = res.tile([64, HPC, S], BF16)
              v_sb = res.tile([P, HPC, 16, DV], BF16)
              # ---------------- Window B: q_b / kv_b + RoPE -> resident qf/kf/v
              with tc.tile_pool(name="pb", bufs=1) as pb, \
                   tc.tile_pool(name="pb2", bufs=2) as pb2, \
                   tc.tile_pool(name="pb1", bufs=2) as pb1, \
                   tc.tile_pool(name="psB", bufs=2, space="PSUM") as psB:
                  qbw_sb = pb.tile([P, 6, 512], BF16)
                  nc.scalar.dma_start(out=qbw_sb, in_=qbw)
                  kvbw_sb = pb.tile([P, 4, 768], BF16)
                  nc.scalar.dma_start(out=kvbw_sb, in_=kvbw)
                  cos_sb = pb.tile([ROPE, S], F32)
                  nc.gpsimd.dma_start(out=cos_sb, in_=cosT)
                  sin_sb = pb.tile([ROPE, S], F32)
                  nc.gpsimd.dma_start(out=sin_sb, in_=sinT)

                  for s in range(NSTRIP):
                      sl = slice(512 * s, 512 * (s + 1))
                      # 5 chunk-pair tiles so matmuls start after 256KB
                      chunks = []
                      for g in range(5):
                          qn_t = pb2.tile([P, 2, 512], BF16, tag=f"qnt{g}",
                                          name=f"qnt{g}")
                          nc.sync.dma_start(
                              out=qn_t,
                              in_=qnT_i[2 * g:2 * g + 2, :, sl].rearrange(
                                  "c p j -> p c j"))
                          chunks.extend([qn_t[:, 0, :], qn_t[:, 1, :]])
                      qn = chunks[:6]
                      kvn = chunks[6:]
                      for hi in range(HPC):
                          for side in range(2):  # 0: q, 1: k/v
                              if side == 0:
                                  wsb, chunks, base, nmt = qbw_sb, qn, 256 * hi, 6
                                  dn, rr = qf_n, qf_r
                              else:
                                  wsb, chunks, base, nmt = kvbw_sb, kvn, 384 * hi, 4
                                  dn, rr = kf_n, kf_r
                              ps_n = psB.tile([P, 512], F32, tag="bn")
                              ps_ror = psB.tile([P, 512], F32, tag="bro")
                              for c in range(nmt):
                                  st, sp = (c == 0), (c == nmt - 1)
                                  nc.tensor.matmul(ps_n, wsb[:, c, base:base + 128],
                                                   chunks[c], start=st, stop=sp)
                                  nc.tensor.matmul(ps_ror,
                                                   wsb[:, c, base + 128:base + 256],
                                                   chunks[c], start=st, stop=sp)
                              nc.scalar.copy(dn[:, hi, sl], ps_n)
                              t1 = pb1.tile([64, 512], F32, tag="t1")
                              nc.vector.tensor_mul(t1, ps_ror[0:64, :],
                                                   cos_sb[:, sl])
                              t2 = pb1.tile([64, 512], F32, tag="t2")
                              nc.vector.tensor_mul(t2, ps_ror[64:128, :],
                                                   sin_sb[:, sl])
                              nc.vector.tensor_add(rr[:, hi, sl], t1, t2)
                              if side == 1:
                                  # v feature-major, then PE transpose
                                  ps_v = psB.tile([P, 512], F32, tag="bv")
                                  for c in range(4):
                                      nc.tensor.matmul(
                                          ps_v, kvbw_sb[:, c, base + 256:base + 384],
                                          chunks[c], start=(c == 0), stop=(c == 3))
                                  v_fm = pb1.tile([P, 512], BF16, tag="vfm")
                                  nc.scalar.copy(v_fm, ps_v)
                                  for t in range(4):
                                      ps_t = psB.tile([P, P], BF16, tag="bt")
                                      nc.tensor.transpose(ps_t, v_fm[:, ts(t, P)],
                                                          ident)
                                      nc.scalar.copy(v_sb[:, hi, 4 * s + t, :], ps_t)

              # ---------------- Window C: attention + o-projection
              with tc.tile_pool(name="pc", bufs=1) as pc, \
                   tc.tile_pool(name="pc2", bufs=2) as pc2, \
                   tc.tile_pool(name="pc3", bufs=3) as pc3, \
                   tc.tile_pool(name="psC", bufs=2, space="PSUM") as psC, \
                   tc.tile_pool(name="psD", bufs=2, space="PSUM") as psD:
                  mask_sb = pc.tile([P, 4, 512], BF16)
                  nc.gpsimd.dma_start(out=mask_sb, in_=masks)
                  ow_sb = pc.tile([P, HPC, H], BF16)
                  nc.gpsimd.dma_start(out=ow_sb, in_=ow)

                  for s in range(NSTRIP):
                      sl = slice(512 * s, 512 * (s + 1))
                      ctx_sb = pc2.tile([P, HPC, 512], BF16, tag="ctx")
                      for hi in range(HPC):
                          ps_ctx = psD.tile([P, 512], F32, tag="ctx")
                          ps_den = psD.tile([1, 512], F32, tag="den")
                          nkc = 4 * s + 4
                          # software-pipelined: att(kc) consumed one
                          # iteration later so PE never waits on Act/Pool.
                          # Diagonal-band chunks shrink to their live q
                          # columns (per-element has_written makes partial
                          # accumulation safe; kc=0 is always full width).
                          pend = []                      # (att, qoff, kc)

                          def flush(last):
                              att_p, qo, kci = pend.pop(0)
                              nc.tensor.matmul(ps_den[:, qo:512], ones_b,
                                               att_p, start=(kci == 0),
                                               stop=last)
                              nc.tensor.matmul(ps_ctx[:, qo:512],
                                               v_sb[:, hi, kci, :], att_p,
                                               start=(kci == 0), stop=last)

                          for kc in range(nkc):
                              diag = kc - 4 * s          # >=0 on diagonal band
                              qoff = 128 * diag if diag > 0 else 0
                              w = 512 - qoff
                              qsl = slice(512 * s + qoff, 512 * (s + 1))
                              ps_sc = psC.tile([P, 512], F32, tag="sc")
                              nc.tensor.matmul(ps_sc[:, 0:w],
                                               kf_n[:, hi, ts(kc, P)],
                                               qf_n[:, hi, qsl],
                                               start=True, stop=False)
                              nc.tensor.matmul(ps_sc[:, 0:w],
                                               kf_r[:, hi, ts(kc, P)],
                                               qf_r[:, hi, qsl],
                                               start=False, stop=True)
                              att_f = pc3.tile([P, 512], BF16, tag="att")
                              att = att_f[:, 0:w]
                              nc.scalar.activation(out=att, in_=ps_sc[:, 0:w],
                                                   func=Exp)
                              if diag >= 0:
                                  nc.gpsimd.tensor_mul(att[:, 0:P], att[:, 0:P],
                                                       mask_sb[:, 0, 0:P])
                              pend.append((att, qoff, kc))
                              if len(pend) > 1:
                                  flush(False)
                          flush(True)
                          rcp = pc2.tile([1, 512], F32, tag="rcp")
                          nc.vector.reciprocal_approx_fast(out=rcp, in_=ps_den)
                          rcp_r = pc2.tile([1, 512], F32R, tag="rcpr")
                          nc.scalar.copy(rcp_r, rcp)
                          # broadcast 1/den across partitions via PE rank-1
                          ps_rb = psC.tile([P, 512], F32, tag="o")
                          nc.tensor.matmul(ps_rb, onesrow_r, rcp_r,
                                           start=True, stop=True)
                          rcp_bc = pc2.tile([P, 512], F32, tag="rbc")
                          nc.scalar.copy(rcp_bc, ps_rb)
                          nc.vector.tensor_mul(ctx_sb[:, hi, :], ps_ctx, rcp_bc)
                      # o-projection for this strip (token-major out)
                      for t in range(4):
                          o_sb = pc2.tile([P, H], BF16, tag="osb")
                          for hs in range(4):
                              ps_o = psC.tile([P, 512], F32, tag="o")
                              for cc in range(HPC):
                                  nc.tensor.matmul(ps_o, ctx_sb[:, cc, ts(t, P)],
                                                   ow_sb[:, cc, ts(hs, 512)],
                                                   start=(cc == 0), stop=(cc == HPC - 1))
                              nc.vector.tensor_copy(o_sb[:, ts(hs, 512)], ps_o)
                          nc.sync.dma_start(
                              out=o_part.rearrange("(T p) h -> p T h", p=P)[:, 4 * s + t, :],
                              in_=o_sb)
    nc.compile()
    return nc


# ---------------------------------------------------------------- NEFF 2
def build_neff2():
    """Expert-parallel MoE, all-bf16 weights/activations (fp32 PSUM accum).

    Per expert slot (2 + shared): 8 gate/up m-tiles of [P,2048]->CAP
    through silu*up*combine, then 16 down h-tiles streamed [P,8,P].
    CAP=512 -> single PSUM strip per tile.
    """
    nc = bacc.Bacc("TRN2", num_devices=NC, debug=False)
    def inp(name, shape):
        return nc.dram_tensor(name, list(shape), BF16, kind="ExternalInput").ap()

    xe = inp("xe", (EPC, 16, P, CAP))        # gathered expert tokens (h2ln)
    gw = inp("gw", (EPC, 8, P, 2048))        # gate lhsT prepack
    uw = inp("uw", (EPC, 8, P, 2048))
    dw = inp("dw", (EPC, 16, P, 8, P))       # down lhsT per h-tile
    wrow = inp("wrow", (EPC, 1, CAP))        # combine weights (row layout)
    h2t = inp("h2t", (16, P, 256))           # my 256 tokens, feature-major
    sgw = inp("sgw", (8, P, 2048))
    suw = inp("suw", (8, P, 2048))
    sdw = inp("sdw", (16, P, 8, P))
    yrT = nc.dram_tensor("yrT", [EPC, 16, P, CAP], BF16,
                         kind="ExternalOutput").ap()
    yshT = nc.dram_tensor("yshT", [16, P, 256], BF16,
                          kind="ExternalOutput").ap()

    with tile.TileContext(nc) as tc:
        with tc.tile_pool(name="px", bufs=2) as px, \
             tc.tile_pool(name="pw", bufs=3) as pw, \
             tc.tile_pool(name="pd", bufs=3) as pd, \
             tc.tile_pool(name="pact", bufs=2) as pact, \
             tc.tile_pool(name="py", bufs=3) as py, \
             tc.tile_pool(name="ps", bufs=2, space="PSUM") as ps, \
             tc.tile_pool(name="psy", bufs=2, space="PSUM") as psy:

            def expert_block(xe_sb, w_bc, gw_i, uw_i, dw_i, y_out, n):
                """One expert's gate/up/silu/down over n token slots."""
                act = pact.tile([P, 8, CAP], BF16, tag="act")
                for t in range(8):
                    g_w = pw.tile([P, 16, P], BF16, tag="gw")
                    nc.sync.dma_start(out=g_w, in_=gw_i[t].rearrange(
                        "p (hc j) -> p hc j", j=P))
                    u_w = pw.tile([P, 16, P], BF16, tag="uw")
                    nc.sync.dma_start(out=u_w, in_=uw_i[t].rearrange(
                        "p (hc j) -> p hc j", j=P))
                    ps_g = ps.tile([P, n], F32, tag="g")
                    ps_u = ps.tile([P, n], F32, tag="u")
                    for hc in range(16):
                        st, sp = (hc == 0), (hc == 15)
                        nc.tensor.matmul(ps_g, g_w[:, hc, :], xe_sb[:, hc, 0:n],
                                         start=st, stop=sp)
                        nc.tensor.matmul(ps_u, u_w[:, hc, :], xe_sb[:, hc, 0:n],
                                         start=st, stop=sp)
                    sil = pw.tile([P, n], F32, tag="sil")
                    nc.scalar.activation(out=sil, in_=ps_g, func=Silu)
                    if w_bc is not None:
                        nc.gpsimd.tensor_mul(sil, sil, w_bc[:, 0:n])
                    nc.vector.tensor_mul(act[:, t, 0:n], sil, ps_u)
                for ht in range(16):
                    d_w = pd.tile([P, 8, P], BF16, tag="dw")
                    nc.sync.dma_start(out=d_w, in_=dw_i[ht])
                    ps_y = psy.tile([P, n], F32, tag="y")
                    for mc in range(8):
                        nc.tensor.matmul(ps_y, d_w[:, mc, :], act[:, mc, 0:n],
                                         start=(mc == 0), stop=(mc == 7))
                    y_sb = py.tile([P, n], BF16, tag="ysb")
                    nc.scalar.copy(y_sb, ps_y)
                    nc.sync.dma_start(out=y_out[ht], in_=y_sb)

            for i in range(EPC):
                xe_sb = px.tile([P, 16, CAP], BF16, tag="xe")
                for qc in range(4):
                    nc.sync.dma_start(
                        out=xe_sb[:, 4 * qc:4 * qc + 4, :],
                        in_=xe[i, 4 * qc:4 * qc + 4].rearrange(
                            "hc p t -> p hc t"))
                w_bc = px.tile([P, CAP], BF16, tag="wbc")
                nc.sync.dma_start(out=w_bc, in_=wrow[i].broadcast_to((P, CAP)))
                expert_block(xe_sb, w_bc, gw[i], uw[i], dw[i], yrT[i], CAP)

            # ---------------- shared expert (my 256 tokens)
            h2_sb = px.tile([P, 16, 256], BF16, tag="h2")
            nc.sync.dma_start(out=h2_sb, in_=h2t.rearrange("hc p t -> p hc t"))
            expert_block(h2_sb, None, sgw, suw, sdw, yshT, 256)
    nc.compile()
    return nc


# ---------------------------------------------------------------- host prep
def _rope_tables():
    inv = 1.0 / (ROPE_BASE ** (np.arange(0, ROPE, 2, dtype=np.float64) / ROPE))
    t = np.arange(S, dtype=np.float64)
    f = t[:, None] * inv[None, :]
    emb = np.concatenate([f, f], axis=-1)          # [S, 64]
    return (np.cos(emb).T.astype(np.float32).copy(),
            np.sin(emb).T.astype(np.float32).copy())


def _lhsT_prepack(wT, mtiles):
    """wT [K, M] -> [P, mtiles, K//P, P]: SBUF-image for resident lhsT tiles."""
    Kd, Md = wT.shape
    assert Md == mtiles * P and Kd % P == 0
    return np.ascontiguousarray(
        wT.reshape(Kd // P, P, mtiles, P).transpose(1, 2, 0, 3))


def _lhsT_prepack2(wT, mtiles):
    """wT [K, M] -> [mtiles, P, K]: per-m-tile contiguous DMA layout.

    Tile t, flattened [P, K] with per-partition layout (hc, j):
    A[t, p, 128*hc + j] = wT[128*hc + p, 128*t + j].
    """
    Kd, Md = wT.shape
    assert Md == mtiles * P and Kd % P == 0
    return np.ascontiguousarray(
        wT.reshape(Kd // P, P, mtiles, P).transpose(2, 1, 0, 3).reshape(
            mtiles, P, Kd))


def _neff0_inputs(x, w):
    bf = ml_dtypes.bfloat16
    ln1 = w["ln1_w"]
    xT = x.T.astype(np.float32)                                   # [H, S]
    WqT = (w["q_a_w"] * ln1[None, :]).T.astype(np.float32)        # [H, QR]
    WkvT = (w["kv_a_w"] * ln1[None, :]).T.astype(np.float32)      # [H, KVR]
    qaw = _lhsT_prepack(WqT, 6).astype(bf)
    kvaw = _lhsT_prepack(WkvT, 4).astype(bf)
    per_core = []
    for c in range(NC):
        rows = slice(256 * c, 256 * (c + 1))
        xTs = np.ascontiguousarray(xT[:, rows].reshape(16, P, 256)
                                   .transpose(1, 0, 2)).astype(bf)
        per_core.append({"xTs": xTs, "qaw": qaw, "kvaw": kvaw})
    return per_core


def _neff1_inputs(w, qnT):
    bf = ml_dtypes.bfloat16
    qb = (w["q_b_w"] * w["q_a_ln"][None, :]).astype(np.float32)   # [NH*DQK, QR]
    kvb = (w["kv_b_w"] * w["kv_a_ln"][None, :]).astype(np.float32)  # [NH*320, KVR]
    sc = 1.0 / math.sqrt(DQK)
    cosT, sinT = _rope_tables()

    masks = np.zeros((P, 4, 512), np.float32)
    pp, jj = np.meshgrid(np.arange(P), np.arange(512), indexing="ij")
    for cl in range(4):
        masks[:, cl, :] = (P * cl + pp <= jj).astype(np.float32)
    masks = masks.astype(bf)

    per_core = []
    for c in range(NC):
        heads = [HPC * c + i for i in range(HPC)]
        # q_b augmented: per head rows [nope 128 | rope 64 | rot 64], scaled by sc
        qrows = []
        for h in heads:
            blk = qb[h * DQK:(h + 1) * DQK] * sc                   # [192, QR]
            nope, rope = blk[:NOPE], blk[NOPE:]
            rot = np.concatenate([-rope[32:], rope[:32]], axis=0)
            qrows.append(np.concatenate([nope, rope, rot], axis=0))  # [256, QR]
        qaug = np.concatenate(qrows, axis=0)                       # [512, QR]
        qbw = np.ascontiguousarray(qaug.T.reshape(6, P, 512).transpose(1, 0, 2))

        kvrows = []
        for h in heads:
            blk = kvb[h * 320:(h + 1) * 320]                       # [320, KVR]
            kn, kr, vv = blk[:NOPE], blk[NOPE:DQK], blk[DQK:]
            krot = np.concatenate([-kr[32:], kr[:32]], axis=0)
            kvrows.append(np.concatenate([kn, kr, krot, vv], axis=0))  # [384, KVR]
        kvaug = np.concatenate(kvrows, axis=0)                     # [768, KVR]
        kvbw = np.ascontiguousarray(kvaug.T.reshape(4, P, 768).transpose(1, 0, 2))

        ocols = np.concatenate([w["o_w"][:, h * DV:(h + 1) * DV] for h in heads],
                               axis=1)                             # [H, 256]
        owp = np.ascontiguousarray(ocols.T.reshape(HPC, P, H).transpose(1, 0, 2))

        per_core.append({
            "qnT": qnT,
            "qbw": qbw.astype(bf), "kvbw": kvbw.astype(bf),
            "ow": owp.astype(bf), "cosT": cosT, "sinT": sinT,
            "masks": masks,
        })
    return per_core


def _rmsn(v, eps=EPS):
    return v / np.sqrt((v * v).mean(-1, keepdims=True) + eps)


def _exact_probs(x, w, tb):
    """fp32 numpy router probs for tokens tb: exact attention rows ->
    exact x2 rows -> exact logits. Borderline-token routing must match the
    fp32 reference; the device's bf16 x2 is too coarse for near-ties."""
    h1 = _rmsn(x) * w["ln1_w"]
    kvl = _rmsn(h1 @ w["kv_a_w"].T) * w["kv_a_ln"]
    kvp = (kvl @ w["kv_b_w"].T).reshape(S, NH, DQK + DV)
    kn, kr, vv = kvp[..., :NOPE], kvp[..., NOPE:DQK], kvp[..., DQK:]
    inv = 1.0 / (ROPE_BASE ** (np.arange(0, ROPE, 2, dtype=np.float64) / ROPE))
    f = np.arange(S, dtype=np.float64)[:, None] * inv[None, :]
    emb = np.concatenate([f, f], axis=-1)
    cos = np.cos(emb).astype(np.float32)[:, None, :]
    sin = np.sin(emb).astype(np.float32)[:, None, :]
    rot = lambda z: np.concatenate([-z[..., 32:], z[..., :32]], axis=-1)
    kf = np.concatenate([kn, kr * cos + rot(kr) * sin], axis=-1)  # [S,NH,192]

    h1b = h1[tb]
    ql = _rmsn(h1b @ w["q_a_w"].T) * w["q_a_ln"]
    qp = (ql @ w["q_b_w"].T).reshape(len(tb), NH, DQK)
    qn_, qr_ = qp[..., :NOPE], qp[..., NOPE:]
    qr_ = qr_ * cos[tb] + rot(qr_) * sin[tb]
    qf = np.concatenate([qn_, qr_], axis=-1)                      # [B,NH,192]

    sc = np.einsum("bhd,khd->bhk", qf, kf) / math.sqrt(DQK)
    keymask = np.arange(S)[None, :] > tb[:, None]                 # [B,S]
    sc = sc - 1e9 * keymask[:, None, :]
    sc -= sc.max(-1, keepdims=True)
    att = np.exp(sc)
    att /= att.sum(-1, keepdims=True)
    ctx = np.einsum("bhk,khd->bhd", att, vv).reshape(len(tb), NH * DV)
    x2r = x[tb] + ctx @ w["o_w"].T
    h2r = _rmsn(x2r) * w["ln2_w"]
    logits = h2r @ w["router_w"].T + w["router_b"][None, :]
    return 1.0 / (1.0 + np.exp(-logits))


def _route(h2ln, w, x):
    """Top-4 routing in numpy; borderline tokens (4th vs 5th prob near-tie)
    get their probs recomputed exactly so the discrete choice matches the
    fp32 reference."""
    logits = h2ln @ w["router_w"].T.astype(np.float32) + w["router_b"][None, :]
    probs = 1.0 / (1.0 + np.exp(-logits))
    sp = np.sort(probs, axis=-1)
    gap = sp[:, -KTOP] - sp[:, -KTOP - 1]
    tb = np.where(gap < 0.01)[0]
    if len(tb):
        probs[tb] = _exact_probs(x, w, tb)
    order = np.argsort(-probs, axis=-1, kind="stable")[:, :KTOP]
    topv = np.take_along_axis(probs, order, axis=-1)
    wts = topv / (topv.sum(-1, keepdims=True) + 1e-9) * SCALE
    return order, wts


def _down_prepack(dwT):
    """dwT [M, H] -> [16, P, 8, P]: per-h-tile lhsT chunks
    out[ht, p, mc, j] = dwT[mc*128+p, ht*128+j]."""
    return np.ascontiguousarray(
        dwT.reshape(8, P, 16, P).transpose(2, 1, 0, 3))


def _neff2_inputs(h2ln, w, order, wts):
    """Expert batches gathered from h2ln (ln2 pre-applied); bf16 payloads."""
    bf = ml_dtypes.bfloat16
    idx_lists, wt_lists = [], []
    for e in range(E):
        tok, kk = np.where(order == e)
        idx_lists.append(tok)
        wt_lists.append(wts[tok, kk])

    h2T = np.ascontiguousarray(h2ln.T).astype(bf)           # [H, S]
    per_core = []
    spill = []                                              # (expert, tok, wt) overflow
    sgw = _lhsT_prepack2(w["sg_w"].T.astype(np.float32), 8).astype(bf)
    suw = _lhsT_prepack2(w["su_w"].T.astype(np.float32), 8).astype(bf)
    sdw = _down_prepack(w["sd_w"].T).astype(bf)

    for c in range(NC):
        xeb = np.zeros((EPC, 16, P, CAP), bf)
        gwb = np.zeros((EPC, 8, P, 2048), bf)
        uwb = np.zeros((EPC, 8, P, 2048), bf)
        dwb = np.zeros((EPC, 16, P, 8, P), bf)
        wrow = np.zeros((EPC, 1, CAP), bf)
        for i in range(EPC):
            e = EPC * c + i
            tok, tw = idx_lists[e], wt_lists[e]
            if len(tok) > CAP:
                spill.append((e, tok[CAP:], tw[CAP:]))
                tok, tw = tok[:CAP], tw[:CAP]
            n = len(tok)
            xeb[i, :, :, :n] = h2T[:, tok].reshape(16, P, n)
            wrow[i, 0, :n] = tw.astype(bf)
            gwb[i] = _lhsT_prepack2(w["gate_w"][e].T.astype(np.float32), 8)
            uwb[i] = _lhsT_prepack2(w["up_w"][e].T.astype(np.float32), 8)
            dwb[i] = _down_prepack(w["down_w"][e].T)
        rows = slice(256 * c, 256 * (c + 1))
        h2tp = np.ascontiguousarray(h2T[:, rows].reshape(16, P, 256))
        per_core.append({
            "xe": xeb, "gw": gwb, "uw": uwb, "dw": dwb, "wrow": wrow,
            "h2t": h2tp, "sgw": sgw, "suw": suw, "sdw": sdw,
        })
    return per_core, idx_lists, wt_lists, spill


def _expert_np(h2ln, idx, wt, w, e):
    """Numpy fallback for capacity-overflow tokens."""
    xg = h2ln[idx]
    g = xg @ w["gate_w"][e].T
    u = xg @ w["up_w"][e].T
    a = (g / (1 + np.exp(-g))) * u
    return (a @ w["down_w"][e].T) * wt[:, None]


# ---------------------------------------------------------------- kernel
def kernel(**inputs):
    w = {k: np.asarray(v, dtype=np.float32) for k, v in inputs.items()}
    x = w["x"][0]                                           # [S, H]

    if "nc0" not in _cache:
        _cache["nc0"] = build_neff0()
    in0 = _neff0_inputs(x, w)
    res0 = _run(_cache["nc0"], in0, "neff0")
    qn_raw = np.concatenate(
        [res0.results[c]["qn_out"] for c in range(NC)],
        axis=2).astype(np.float32)                        # [10, P, S]
    msq = np.concatenate([res0.results[c]["msq_out"] for c in range(NC)],
                         axis=2)                          # [2, 1, S]
    arow = ((x * x).sum(-1) / H + EPS).astype(np.float32)  # [S]
    sig_q = 1.0 / np.sqrt(msq[0, 0] / QR + EPS * arow)     # [S]
    sig_kv = 1.0 / np.sqrt(msq[1, 0] / KVR + EPS * arow)
    qn_raw[:6] *= sig_q[None, None, :]
    qn_raw[6:] *= sig_kv[None, None, :]
    qnT = qn_raw.astype(ml_dtypes.bfloat16)

    if "nc1" not in _cache:
        _cache["nc1"] = build_neff1()
    nc1 = _cache["nc1"]
    in1 = _neff1_inputs(w, qnT)
    res1 = _run(nc1, in1, "neff1")
    o_sum = np.zeros((S, H), np.float32)
    for c in range(NC):
        o_sum += res1.results[c]["o_part"].astype(np.float32)
    x2 = x + o_sum

    r2 = 1.0 / np.sqrt((x2 * x2).mean(-1, keepdims=True) + EPS)
    h2 = (x2 * r2).astype(np.float32)                       # rmsnorm w/o ln2
    h2ln = h2 * w["ln2_w"][None, :]
    order, wts = _route(h2ln, w)

    if "nc2" not in _cache:
        _cache["nc2"] = build_neff2()
    nc2 = _cache["nc2"]
    in2, idx_lists, wt_lists, spill = _neff2_inputs(h2ln, w, order, wts)
    res2 = _run(nc2, in2, "neff2")

    out = x2.copy()
    for c in range(NC):
        r = res2.results[c]
        for i in range(EPC):
            e = EPC * c + i
            tok = idx_lists[e][:CAP]
            ye = r["yrT"][i].reshape(H, CAP).T.astype(np.float32)  # [CAP, H]
            out[tok] += ye[:len(tok)]
        out[256 * c:256 * (c + 1)] += r["yshT"].reshape(H, 256).T.astype(
            np.float32)
    for e, tok, tw in spill:
        out[tok] += _expert_np(h2ln, tok, tw, w, e)
    return out.reshape(1, S, H).astype(np.float32)

